# revision 25
# baseline (speedup 1.0000x reference)
"""2-layer GAT (gnn_message_passing) on 8 TRN2 NeuronCores.

Strategy (graph/data parallel, per sharding hint):
  - Nodes are partitioned across 8 ranks (6250 dst nodes each). Each rank owns
    the segment-softmax + aggregation for its destination nodes.
  - Per layer, every rank computes the projected features (h = x @ W,
    attention source/dest logits al/ar fused into the same matmul via an
    augmented RHS) for ITS OWN nodes, writes them as rows of a gather table
    (768B rows for layer 1: 256 bf16 h + 8 f32 al; 256B rows for layer 2),
    then an AllGather replicates the full table to every rank.
  - Edge stage: destinations are degree-sorted and packed into tiles of 128
    (dst on partitions); consecutive tiles are grouped with a UNIFORM padded
    slot count per group so that the whole group is one rectangular grid
    [128, G, D, ...] and every vector/scalar op covers the full group in a
    single instruction. Source rows are fetched with dma_gather (SWDGE
    indexed gather). Since gather indices are int16 (max 32767) and the
    table has ~60k rows, rows are addressed through an even/odd pair view
    (idx = row//2): each group issues one even-window and one odd-window
    gather, so every edge needs a parity side.
  - Slot-padding control: per-tile slot counts are max(c_even), max(c_odd)
    over the (degree-sorted) tile's dsts. A host-side balancer chooses each
    source node's row parity to balance every dst's in-edge parity split;
    residual "blocker" sources are DUPLICATED (their row is copied once more
    at the opposite parity at the end of the owner rank's chunk), making all
    their edges per-edge flexible. This brings padded slots within ~2% of
    the true floor (max in-degree per tile). L1 and L2 use separate group
    schedules: L2 rows are 4x smaller so its groups merge ~3x more tiles.
  - Segment softmax is all free-dim math: e = leakyrelu(al_src + ar_dst) on
    the slot grid, p = exp(e) (no max-subtract needed at these magnitudes;
    mathematically identical), denom = free-dim reduce, normalization applied
    AFTER aggregation (divide the aggregated sums by denom).
  - Aggregation: msg = p (broadcast over channels by doubling copies on the
    otherwise-idle Scalar engine) * h_src, then a pairwise tree of wide
    tensor adds along the slot dim.
  - Padding slots read a sentinel table row (h = 0, al = -1e30 -> p = 0).

The full output is assembled on the host from the 8 per-rank outputs
(undoing the degree-sort permutation).
"""

import sys
from contextlib import ExitStack
from dataclasses import dataclass

import numpy as np

for _p in ("/opt/trn_rl_repo",):
    if _p not in sys.path:
        sys.path.insert(0, _p)

import concourse.bass as bass
import concourse.bacc as bacc
import concourse.mybir as mybir
import concourse.tile as tile
from concourse import bass_utils


F32 = mybir.dt.float32
BF16 = mybir.dt.bfloat16
I16 = mybir.dt.int16
AL_SENT = -1.0e30
Alu = mybir.AluOpType
Act = mybir.ActivationFunctionType


@dataclass
class Cfg:
    N: int = 50000
    E: int = 500000          # edges before self-loops
    F_IN: int = 128
    HID: int = 32
    HEADS: int = 8
    OUT: int = 64
    NEG: float = 0.2
    R: int = 8
    SLOT1: int = 36          # L1 max uniform slots per gather group
    MSG1: int = 28           # L1 max slots per region
    SLOT2: int = 104         # L2 caps (rows 4x smaller; pools shared with L1)
    MSG2: int = 96
    NDUP: int = 1664         # duplicate rows per rank (multiple of 128, even)
    TA1: int = 0             # AG1 split tile boundary (0 = no split)
    TA2: int = 34            # AG2 split tile boundary (0 = no split)

    @property
    def HC1(self):
        return self.HEADS * self.HID     # 256

    @property
    def NPR(self):
        return self.N // self.R

    @property
    def T(self):
        return (self.NPR + 127) // 128   # dst tiles per rank

    @property
    def NT(self):
        return self.T * 128

    @property
    def CHUNK(self):
        # sentinel + staged rows (NT >= NPR) + duplicate rows; must be odd
        return 1 + self.NT + self.NDUP

    @property
    def TROWS(self):
        return self.R * self.CHUNK

    @property
    def ROW1(self):
        return 384                       # bf16 elems: 256 h + 16 (8xf32 al) + pad

    @property
    def ROW2(self):
        return 128                       # bf16 elems: 64 h2 + 2 (1xf32 al2) + pad


@dataclass
class Sched:
    perm: np.ndarray          # [R, NPR] perm[r][pos] = global node id
    groups1: list             # L1 groups: (t0, t1, DL, DH)
    call_cols1: list          # per group, section-relative (lo0, lnc, hi0, hnc)
    groups2: list             # L2 groups
    call_cols2: list
    idx16: np.ndarray         # [R, 128, TOTCOL] int16 (L1 | L2 | dup)
    off2: int                 # column offset of the L2 section
    offd: int                 # column offset of the dup section
    ta2: int = 0              # AG2 split tile boundary (0 = no split)
    sent_hi2: int = 0         # odd-pad sentinel idx for the L2 table


def _pack_idx(vals: np.ndarray) -> np.ndarray:
    """int32 row-idx values -> the [128, n/16] int16 SWDGE index layout."""
    assert vals.min() >= 0 and vals.max() < 32768, (vals.min(), vals.max())
    return np.tile(vals.astype(np.int16).reshape(-1, 16).T, (8, 1))


def build_schedule(cfg: Cfg, src: np.ndarray, dst: np.ndarray) -> Sched:
    N, R, NPR, T = cfg.N, cfg.R, cfg.NPR, cfg.T
    CHUNK, NT = cfg.CHUNK, cfg.NT
    assert CHUNK % 2 == 1 and 4 * CHUNK < 32768
    deg = np.bincount(dst, minlength=N).astype(np.int64)

    # ---- global degree-sorted tiles (1024 nodes per global tile) ----
    gorder = np.argsort(-deg, kind="stable")
    gtile = np.empty(N, np.int64)
    for t in range(T):
        gtile[gorder[t * 1024:(t + 1) * 1024]] = t
    tile_of_dst = gtile[dst]
    maxdeg_t = np.array([max(1, deg[gorder[t * 1024:(t + 1) * 1024]].max())
                         for t in range(T)])

    eorder = np.argsort(src, kind="stable")
    s_sorted = src[eorder]
    d_sorted = dst[eorder]
    starts = np.searchsorted(s_sorted, np.arange(N + 1))

    # ---- parity balancing: conflict-free vectorized greedy ----
    rng = np.random.default_rng(12345)
    parity = np.zeros(N, np.int8)
    tile_nodes = []
    for t in range(T):
        nodes = gorder[t * 1024:(t + 1) * 1024]
        tile_nodes.append(nodes)
        p = np.zeros(len(nodes), np.int8)
        p[:len(nodes) // 2] = 1
        rng.shuffle(p)
        parity[nodes] = p

    c_e = np.zeros(N, np.int32)
    c_o = np.zeros(N, np.int32)
    pe = parity[src]
    np.add.at(c_e, dst[pe == 0], 1)
    np.add.at(c_o, dst[pe == 1], 1)

    # alternate the ceil side per tile so parity peaks (and hence dup-copy
    # parity demand) split ~evenly between the even and odd windows
    ceil_half = np.ceil((maxdeg_t + 1) / 2).astype(np.int64)
    Te = np.where(np.arange(T) % 2 == 0, ceil_half, (maxdeg_t + 1) - ceil_half)
    To = (maxdeg_t + 1) - Te
    TeD = Te[tile_of_dst]
    ToD = To[tile_of_dst]
    imb = np.zeros(T, np.int64)
    CAP, W = 12, 8.0

    def pen(c, Tt):
        return np.where(c > Tt, W ** np.minimum(c - Tt, 6), 0.0)

    for rnd in range(120):
        ceD = c_e[dst]
        coD = c_o[dst]
        d_eo = (pen(ceD - 1, TeD) - pen(ceD, TeD)) + (pen(coD + 1, ToD) - pen(coD, ToD))
        d_oe = (pen(coD - 1, ToD) - pen(coD, ToD)) + (pen(ceD + 1, TeD) - pen(ceD, TeD))
        cum_eo = np.concatenate([[0.], np.cumsum(d_eo[eorder])])
        cum_oe = np.concatenate([[0.], np.cumsum(d_oe[eorder])])
        g_eo = -(cum_eo[starts[1:]] - cum_eo[starts[:-1]])
        g_oe = -(cum_oe[starts[1:]] - cum_oe[starts[:-1]])
        gain = np.where(parity == 0, g_eo, g_oe)
        cand = np.where(gain > 1e-9)[0]
        if len(cand) == 0:
            break
        cand = cand[np.argsort(-gain[cand])]
        dirty = np.zeros(N, bool)
        napp = 0
        for u in cand:
            ds = d_sorted[starts[u]:starts[u + 1]]
            if dirty[ds].any():
                continue
            t = gtile[u]
            delta = 1 if parity[u] == 0 else -1
            if abs(imb[t] + delta) > CAP:
                continue
            dirty[ds] = True
            imb[t] += delta
            napp += 1
            if parity[u] == 0:
                c_e[ds] -= 1
                c_o[ds] += 1
                parity[u] = 1
            else:
                c_o[ds] -= 1
                c_e[ds] += 1
                parity[u] = 0
        if napp == 0:
            break
    # repair per-tile parity balance to exact 50/50
    for t in range(T):
        while imb[t] != 0:
            nodes = tile_nodes[t]
            want = 1 if imb[t] > 0 else 0
            pool = nodes[parity[nodes] == want]
            ceD = c_e[dst]
            coD = c_o[dst]
            if want == 1:
                dpe = (pen(coD - 1, ToD) - pen(coD, ToD)) + (pen(ceD + 1, TeD) - pen(ceD, TeD))
            else:
                dpe = (pen(ceD - 1, TeD) - pen(ceD, TeD)) + (pen(coD + 1, ToD) - pen(coD, ToD))
            cum = np.concatenate([[0.], np.cumsum(dpe[eorder])])
            gg = -(cum[starts[1:]] - cum[starts[:-1]])
            bu = pool[np.argmax(gg[pool])]
            ds = d_sorted[starts[bu]:starts[bu + 1]]
            if want == 1:
                c_o[ds] -= 1
                c_e[ds] += 1
                parity[bu] = 0
                imb[t] -= 1
            else:
                c_e[ds] -= 1
                c_o[ds] += 1
                parity[bu] = 1
                imb[t] += 1

    # ---- duplicate "blocker" sources until forced maxima reach the floor ----
    dup = np.zeros(N, bool)
    max_dups = (cfg.NDUP // 2 - 32) * 2 * R  # conservative global budget

    def forced_stats():
        f_e = np.zeros(N, np.int32)
        f_o = np.zeros(N, np.int32)
        m = ~dup[src]
        pp = parity[src]
        np.add.at(f_e, dst[m & (pp == 0)], 1)
        np.add.at(f_o, dst[m & (pp == 1)], 1)
        FE = np.zeros(T, np.int64)
        FO = np.zeros(T, np.int64)
        np.maximum.at(FE, tile_of_dst, f_e[dst])
        np.maximum.at(FO, tile_of_dst, f_o[dst])
        return f_e, f_o, np.maximum(FE, 1), np.maximum(FO, 1)

    for it in range(200):
        f_e, f_o, FE, FO = forced_stats()
        bind = (FE + FO) > maxdeg_t
        if not bind.any() or dup.sum() >= max_dups:
            break
        peak_e = (bind[tile_of_dst] & (f_e[dst] == FE[tile_of_dst])
                  & (parity[src] == 0) & ~dup[src])
        peak_o = (bind[tile_of_dst] & (f_o[dst] == FO[tile_of_dst])
                  & (parity[src] == 1) & ~dup[src])
        sc = np.zeros(N, np.int64)
        np.add.at(sc, src[peak_e | peak_o], 1)
        order = np.argsort(-sc)
        take = order[sc[order] > 0][:200]
        if len(take) == 0:
            break
        dup[take] = True
    f_e, f_o, FE, FO = forced_stats()

    # ---- per-tile slot budgets + flexible (dup-sourced) edge assignment ----
    B_t = np.maximum(FE + FO, maxdeg_t)
    mid = np.where(np.arange(T) % 2 == 0, np.ceil(B_t / 2),
                   np.floor(B_t / 2)).astype(np.int64)
    Te_t = np.clip(mid, FE, B_t - FO)
    flex_cnt = np.zeros(N, np.int32)
    np.add.at(flex_cnt, dst[dup[src]], 1)
    TeN = Te_t[gtile]                        # per-dst lo budget
    lo_cnt = f_e + np.minimum(flex_cnt, np.maximum(0, TeN - f_e)).astype(np.int32)
    assert (lo_cnt <= TeN).all()
    assert ((deg - lo_cnt) <= (B_t - Te_t)[gtile]).all()

    # ---- placement: assign nodes to (rank, position) honoring parity ----
    perm = np.empty((R, NPR), np.int64)
    rank_of = np.empty(N, np.int64)
    sortpos = np.empty(N, np.int64)
    for t in range(T):
        nodes = tile_nodes[t]
        k = len(nodes) // R
        # dup'd nodes first within each parity class, then stride-8 interleave
        # across ranks so each rank gets an equal share of dup copies
        ev_nodes = nodes[parity[nodes] == 0]
        od_nodes = nodes[parity[nodes] == 1]
        ev_nodes = ev_nodes[np.argsort(~dup[ev_nodes], kind="stable")]
        od_nodes = od_nodes[np.argsort(~dup[od_nodes], kind="stable")]
        for r in range(R):
            pos = t * 128 + np.arange(k)
            pp = (r + 1 + pos) % 2            # row parity of each position
            ev_pos = pos[pp == 0]
            od_pos = pos[pp == 1]
            a = ev_nodes[r::R]
            b = od_nodes[r::R]
            assert len(a) == len(ev_pos) and len(b) == len(od_pos), (t, r)
            perm[r, ev_pos] = a
            perm[r, od_pos] = b
            rank_of[a] = r
            rank_of[b] = r
            sortpos[a] = ev_pos
            sortpos[b] = od_pos

    row_of = rank_of * CHUNK + 1 + sortpos
    assert ((row_of % 2) == parity).all()

    # ---- dup row placement: copy at opposite parity in owner rank chunk ----
    dup_row = np.full(N, -1, np.int64)
    dup_vals = np.zeros((R, cfg.NDUP), np.int64)
    for r in range(R):
        dn = np.where(dup & (rank_of == r))[0]
        need_even = dn[parity[dn] == 1]
        need_odd = dn[parity[dn] == 0]
        slots = np.arange(cfg.NDUP)
        spar = (r + 1 + NT + slots) % 2
        ev_slots = slots[spar == 0]
        od_slots = slots[spar == 1]
        assert len(need_even) <= len(ev_slots) and len(need_odd) <= len(od_slots), (
            r, len(need_even), len(need_odd))
        for nn, ss in ((need_even, ev_slots), (need_odd, od_slots)):
            rows = r * CHUNK + 1 + NT + ss[:len(nn)]
            dup_row[nn] = rows
            dup_vals[r, ss[:len(nn)]] = 1 + sortpos[nn]
    row_even = np.where(parity == 0, row_of, dup_row)
    row_odd = np.where(parity == 1, row_of, dup_row)

    # ---- per-edge side + slot assignment ----
    dkey = rank_of[dst] * NPR + sortpos[dst]
    side = np.where(dup[src], -1, parity[src]).astype(np.int64)  # -1 = flex
    order0 = np.lexsort((side, dkey))
    s_side = side[order0]
    s_dkey = dkey[order0]
    cnt = np.bincount(dkey, minlength=R * NPR)
    start = np.concatenate([[0], np.cumsum(cnt)])[:-1]
    pos_in = np.arange(len(order0)) - start[s_dkey]
    nflo = np.bincount(dkey[side == 0], minlength=R * NPR)
    lo_cnt_d = lo_cnt[dst[order0]]
    flex_lo_quota = lo_cnt_d - nflo[s_dkey]
    is_flex = s_side == -1
    new_side = np.where(is_flex, np.where(pos_in < flex_lo_quota, 0, 1), s_side)

    order1 = np.lexsort((new_side, s_dkey))
    f_side = new_side[order1]
    f_dkey = s_dkey[order1]
    f_edge = order0[order1]
    pos_f = np.arange(len(order1)) - start[f_dkey]
    nlo_f = np.bincount(f_dkey[f_side == 0], minlength=R * NPR)
    slot = np.where(f_side == 0, pos_f, pos_f - nlo_f[f_dkey])
    f_src = src[f_edge]
    f_row = np.where(f_side == 0, row_even[f_src], row_odd[f_src])
    assert (f_row >= 0).all()
    assert ((f_row % 2) == f_side).all()

    # per-tile slot maxima (shared by all ranks)
    D_lo = np.zeros(T, np.int64)
    D_hi = np.zeros(T, np.int64)
    nlo_g = nlo_f.reshape(R, NPR)
    nhi_g = (cnt - nlo_f).reshape(R, NPR)
    for t in range(T):
        sl = slice(t * 128, min((t + 1) * 128, NPR))
        D_lo[t] = max(1, nlo_g[:, sl].max())
        D_hi[t] = max(1, nhi_g[:, sl].max())

    def make_groups(SLOT_CAP, MSG_CAP):
        groups = []
        t0 = 0
        while t0 < T:
            t1, DL, DH = t0, 0, 0
            while t1 < T:
                nDL = max(DL, int(D_lo[t1]))
                nDH = max(DH, int(D_hi[t1]))
                G1 = t1 - t0 + 1
                if (G1 * (nDL + nDH) > SLOT_CAP
                        or G1 * max(nDL, nDH) > MSG_CAP):
                    break
                DL, DH, t1 = nDL, nDH, t1 + 1
            assert t1 > t0, (t0, D_lo[t0], D_hi[t0])
            groups.append((t0, t1, DL, DH))
            t0 = t1
        return groups

    def layout(groups):
        """Section-relative column layout. Returns (call_cols, pos_base, ncols)."""
        call_cols = []
        pos_base = np.zeros((T, 2), np.int64)
        col = 0
        for (t0, t1, DL, DH) in groups:
            G = t1 - t0
            lo0 = col
            for g, t in enumerate(range(t0, t1)):
                pos_base[t, 0] = col * 16 + g * DL * 128
            col += G * DL * 8
            hi0 = col
            for g, t in enumerate(range(t0, t1)):
                pos_base[t, 1] = col * 16 + g * DH * 128
            col += G * DH * 8
            call_cols.append((lo0, G * DL * 8, hi0, G * DH * 8))
        return call_cols, pos_base, col

    groups1 = make_groups(cfg.SLOT1, cfg.MSG1)
    groups2 = make_groups(cfg.SLOT2, cfg.MSG2)

    # ---- L2 table row mapping (two-piece layout for the split AllGather) ----
    # piece A = all ranks' chunk rows [0, mA); piece B = chunk rows
    # [mA-1, CHUNK) per rank (leading duplicate of row mA-1 keeps mB' odd so
    # that row parity matches the chunk parity used for side assignment).
    if cfg.TA2:
        ta2 = 0
        for (g_t0, g_t1, _dl, _dh) in groups1:
            if g_t1 >= cfg.TA2:
                ta2 = g_t1
                break
        mA = 1 + ta2 * 128
        mBp = CHUNK - mA + 1
        assert mA % 2 == 1 and mBp % 2 == 1

        def row2_of(c, r):
            return np.where(c < mA, r * mA + c,
                            R * mA + r * mBp + c - mA + 1)
        sent_hi2 = mA // 2  # rank-1 sentinel: row2 = mA (odd)
    else:
        ta2 = 0

        def row2_of(c, r):
            return r * CHUNK + c
        sent_hi2 = (CHUNK - 1) // 2
    chunk_row = row_of - rank_of * CHUNK
    row2_node = row2_of(chunk_row, rank_of)
    dup_chunk_row = np.where(dup_row >= 0, dup_row % CHUNK, 0)
    dup_row2 = np.where(dup_row >= 0,
                        row2_of(dup_chunk_row, rank_of), -1)
    assert ((row2_node % 2) == parity).all()
    assert ((dup_row2[dup] % 2) == (1 - parity[dup])).all()
    row_even2 = np.where(parity == 0, row2_node, dup_row2)
    row_odd2 = np.where(parity == 1, row2_node, dup_row2)
    f_row2 = np.where(f_side == 0, row_even2[f_src], row_odd2[f_src])
    assert (f_row2 >= 0).all() and ((f_row2 % 2) == f_side).all()
    call_cols1, pos_base1, ncol1 = layout(groups1)
    call_cols2, pos_base2, ncol2 = layout(groups2)
    ndup_cols = cfg.NDUP // 16
    TOTCOL = ncol1 + ncol2 + ndup_cols

    SENT_LO = 0                 # even pad: rank-0 sentinel row 0
    SENT_HI = (CHUNK - 1) // 2  # odd pad: rank-1 sentinel row CHUNK (odd)

    e_rank = f_dkey // NPR
    e_pos = f_dkey % NPR
    e_tile = e_pos // 128
    e_part = e_pos % 128
    idx16 = np.empty((R, 128, TOTCOL), np.int16)
    for r in range(R):
        m = (e_rank == r)
        mlo = m & (f_side == 0)
        mhi = m & (f_side == 1)
        sections = []
        for (groups, call_cols, pos_base, ncol, rows, s_hi) in (
                (groups1, call_cols1, pos_base1, ncol1, f_row, SENT_HI),
                (groups2, call_cols2, pos_base2, ncol2, f_row2, sent_hi2)):
            vals = np.empty(ncol * 16, np.int32)
            for (lc0, lnc, hc0, hnc) in call_cols:
                vals[lc0 * 16:(lc0 + lnc) * 16] = SENT_LO
                vals[hc0 * 16:(hc0 + hnc) * 16] = s_hi
            p_lo = pos_base[e_tile[mlo], 0] + slot[mlo] * 128 + e_part[mlo]
            vals[p_lo] = rows[mlo] // 2
            p_hi = pos_base[e_tile[mhi], 1] + slot[mhi] * 128 + e_part[mhi]
            vals[p_hi] = (rows[mhi] - 1) // 2
            sections.append(_pack_idx(vals))
        sections.append(_pack_idx(dup_vals[r]))
        idx16[r] = np.concatenate(sections, axis=1)

    return Sched(perm=perm, groups1=groups1, call_cols1=call_cols1,
                 groups2=groups2, call_cols2=call_cols2, idx16=idx16,
                 off2=ncol1, offd=ncol1 + ncol2, ta2=ta2, sent_hi2=sent_hi2)


def _bc(ap, shape):
    """broadcast an AP to shape (step-0 dims)"""
    return ap.broadcast_to(list(shape))


def build_program(cfg: Cfg, sch: Sched):
    """Build the single SPMD Bass program. Returns nc."""
    nc = bacc.Bacc("TRN2", target_bir_lowering=False, debug=False,
                   num_devices=cfg.R, num_swdge_queues=2)
    T, NPR, CHUNK, TROWS, NT = cfg.T, cfg.NPR, cfg.CHUNK, cfg.TROWS, cfg.NT
    HC1, H, HID, OUT = cfg.HC1, cfg.HEADS, cfg.HID, cfg.OUT
    ROW1, ROW2, NDUP = cfg.ROW1, cfg.ROW2, cfg.NDUP
    TOTCOL = sch.idx16.shape[2]

    # ---- I/O ----
    xT = nc.dram_tensor("xT", [cfg.F_IN, NT], F32, kind="ExternalInput")
    idxs_d = nc.dram_tensor("idxs", [128, TOTCOL], I16, kind="ExternalInput")
    W1_d = nc.dram_tensor("W1", [cfg.F_IN, HC1], F32, kind="ExternalInput")
    W1T_d = nc.dram_tensor("W1T", [HC1, cfg.F_IN], F32, kind="ExternalInput")
    A1s_d = nc.dram_tensor("A1s", [HC1, H], F32, kind="ExternalInput")
    A1d_d = nc.dram_tensor("A1d", [HC1, H], F32, kind="ExternalInput")
    B1_d = nc.dram_tensor("B1rep", [128, HC1], F32, kind="ExternalInput")
    W2_d = nc.dram_tensor("W2", [HC1, OUT], F32, kind="ExternalInput")
    W2T_d = nc.dram_tensor("W2T", [OUT, HC1], F32, kind="ExternalInput")
    a2s_d = nc.dram_tensor("a2s", [OUT, 1], F32, kind="ExternalInput")
    a2d_d = nc.dram_tensor("a2d", [OUT, 1], F32, kind="ExternalInput")
    B2_d = nc.dram_tensor("B2rep", [128, OUT], F32, kind="ExternalInput")
    out_d = nc.dram_tensor("out", [NT, OUT], F32, kind="ExternalOutput")

    KC = HC1 // 128   # contraction chunks over HC1 (2)

    with tile.TileContext(nc) as tc, ExitStack() as ctx:
        dram = ctx.enter_context(tc.tile_pool(name="dram", bufs=1, space="DRAM"))
        const = ctx.enter_context(tc.tile_pool(name="const", bufs=1))
        psum = ctx.enter_context(tc.tile_pool(name="psum", bufs=2, space="PSUM"))

        # DRAM scratch
        chunk1 = dram.tile([CHUNK, ROW1], BF16)
        table1 = dram.tile([TROWS, ROW1], BF16, addr_space="Shared")
        chunk2 = dram.tile([CHUNK, ROW2], BF16)
        # table2 is NOT in the Shared address space: Shared DRAM allows a
        # single writer instruction, and the AG2 overlap needs two partial
        # AllGathers writing disjoint (contiguous) row ranges of the
        # two-piece layout (one extra row per rank in piece B)
        TROWS2 = cfg.R * (CHUNK + 1) if sch.ta2 else TROWS
        table2 = dram.tile([TROWS2, ROW2], BF16,
                           addr_space="Shared" if not sch.ta2 else "Local")
        h1d = dram.tile([NT, HC1], BF16)

        # ---- persistent constants ----
        # idx SBUF is swapped per layer: [active-layer cols | dup cols]
        MAXCOL = max(sch.off2, sch.offd - sch.off2)
        idx_s = const.tile([128, MAXCOL], I16, tag="idx")
        nc.sync.dma_start(idx_s[:, 0:sch.off2], idxs_d[:, 0:sch.off2])
        dup_s = const.tile([128, NDUP // 16], I16, tag="idxdup")
        nc.sync.dma_start(dup_s[:, :], idxs_d[:, sch.offd:sch.offd + NDUP // 16])
        RHS1 = const.tile([128, HC1 + 2 * H], F32, tag="rhs1")
        nc.sync.dma_start(RHS1[:, 0:HC1], W1_d[:, :])
        B1_s = const.tile([128, HC1], BF16, tag="b1")
        B1f_s = const.tile([128, HC1], F32, tag="b1f")
        nc.sync.dma_start(B1f_s[:, :], B1_d[:, :])
        nc.vector.tensor_copy(B1_s[:, :], B1f_s[:, :])
        B2_s = const.tile([128, OUT], F32, tag="b2")
        nc.sync.dma_start(B2_s[:, :], B2_d[:, :])
        arL = const.tile([128, T, H], F32, tag="arL")
        nc.vector.memset(arL[:, :, :], 0.0)
        ar2L = const.tile([128, T, 1], F32, tag="ar2L")
        nc.vector.memset(ar2L[:, :, :], 0.0)
        RHS2 = const.tile([128, KC, OUT + 2], BF16, tag="rhs2")
        nc.gpsimd.dma_start(RHS2[:, :, 0:OUT],
                            W2_d.ap().rearrange("(k p) c -> p k c", p=128))
        pS1 = const.tile([128, T, H], BF16, tag="pS1")
        pS2 = const.tile([128, T, 1], BF16, tag="pS2")

        # ================= phase 1: projection + table 1 ====================
        with tc.tile_pool(name="ph1", bufs=1) as ph1:
            xT_s = ph1.tile([128, NT], F32, tag="xT")
            nc.sync.dma_start(xT_s[:, :], xT[:, :])
            W1T_s = ph1.tile([128, KC, 128], F32, tag="w1t")
            nc.sync.dma_start(W1T_s[:, :, :],
                              W1T_d.ap().rearrange("(k p) f -> p k f", p=128))
            A1s_s = ph1.tile([128, KC, H], F32, tag="a1s")
            nc.sync.dma_start(A1s_s[:, :, :],
                              A1s_d.ap().rearrange("(k p) h -> p k h", p=128))
            A1d_s = ph1.tile([128, KC, H], F32, tag="a1d")
            nc.sync.dma_start(A1d_s[:, :, :],
                              A1d_d.ap().rearrange("(k p) h -> p k h", p=128))
            W2T_s = ph1.tile([OUT, HC1], F32, tag="w2t")
            nc.sync.dma_start(W2T_s[:, :], W2T_d[:, :])
            a2s_s = ph1.tile([OUT, 1], F32, tag="a2s")
            nc.sync.dma_start(a2s_s[:, :], a2s_d[:, :])
            a2d_s = ph1.tile([OUT, 1], F32, tag="a2d")
            nc.sync.dma_start(a2d_s[:, :], a2d_d[:, :])

            # fold attention vectors into projection RHS
            for (dst_off, A_s) in ((HC1, A1s_s), (HC1 + H, A1d_s)):
                ps = psum.tile([128, H], F32, tag="wprep")
                for k in range(KC):
                    nc.tensor.matmul(ps[:, :], W1T_s[:, k, :], A_s[:, k, :],
                                     start=(k == 0), stop=(k == KC - 1))
                nc.vector.tensor_copy(RHS1[:, dst_off:dst_off + H], ps[:, :])
            for (dst_off, a_s) in ((OUT, a2s_s), (OUT + 1, a2d_s)):
                for k in range(KC):
                    ps = psum.tile([128, 1], F32, tag="wprep2")
                    nc.tensor.matmul(ps[:, :], W2T_s[:, k * 128:(k + 1) * 128],
                                     a_s[:, :], start=True, stop=True)
                    nc.vector.tensor_copy(RHS2[:, k, dst_off:dst_off + 1],
                                          ps[:, :])

            # sentinel row -> chunk row 0 (h = 0, al = -1e30); written first
            # so the first partial AllGather can cover it
            sent1 = ph1.tile([1, ROW1], BF16, tag="sent1")
            nc.vector.memset(sent1[:, :], 0.0)
            nc.vector.memset(sent1[:, HC1:HC1 + 2 * H].bitcast(F32), AL_SENT)
            nc.sync.dma_start(chunk1[0:1, :], sent1[:, :])
            tstage = ph1.tile([128, T, ROW1], BF16, tag="tstage1")
            nc.vector.memset(tstage[:, :, :], 0.0)
            TA1 = cfg.TA1
            for t in range(T):
                ps = psum.tile([128, HC1 + 2 * H], F32, tag="proj1")
                nc.tensor.matmul(ps[:, :], xT_s[:, t * 128:(t + 1) * 128],
                                 RHS1[:, :], start=True, stop=True)
                nc.scalar.copy(tstage[:, t, 0:HC1], ps[:, 0:HC1])
                al_view = tstage[:, t, HC1:HC1 + 2 * H].bitcast(F32)
                nc.vector.tensor_copy(al_view[:, :], ps[:, HC1:HC1 + H])
                nc.vector.tensor_copy(arL[:, t, :],
                                      ps[:, HC1 + H:HC1 + 2 * H])
                if TA1 and t == TA1 - 1:
                    nc.sync.dma_start(
                        chunk1[1:1 + TA1 * 128, :].rearrange(
                            "(t p) c -> p t c", p=128),
                        tstage[:, 0:TA1, :])
                    nc.gpsimd.collective_compute(
                        "AllGather", Alu.bypass,
                        replica_groups=[list(range(cfg.R))],
                        ins=[chunk1[0:1 + TA1 * 128, :].opt()],
                        outs=[table1[:, :].rearrange(
                            "(r c) w -> r c w", r=cfg.R)[
                            :, 0:1 + TA1 * 128, :].opt()])
            # dense self-loop weights: pS1 = exp(leakyrelu(al + ar))
            eS = ph1.tile([128, T, H], F32, tag="eS")
            alL = tstage[:, 0:T, HC1:HC1 + 2 * H].bitcast(F32)
            nc.vector.tensor_add(eS[:, :, :], alL, arL[:, :, :])

            nc.vector.scalar_tensor_tensor(
                eS[:, :, :], eS[:, :, :], cfg.NEG, eS[:, :, :],
                op0=Alu.mult, op1=Alu.max)
            nc.scalar.activation(pS1[:, :, :], eS[:, :, :], Act.Exp)
            TA1 = cfg.TA1
            nc.sync.dma_start(
                chunk1[1 + TA1 * 128:1 + NT, :].rearrange(
                    "(t p) c -> p t c", p=128),
                tstage[:, TA1:T, :])
            # duplicate rows: indexed re-fetch of own chunk rows, append
            dupb = ph1.tile([128, NDUP // 128, ROW1], BF16, tag="dupb")
            nc.gpsimd.dma_gather(
                dupb[:, :, :], chunk1[0:1 + NT, :],
                dup_s[:, :],
                num_idxs=NDUP, num_idxs_reg=NDUP,
                elem_size=ROW1, single_packet=False, queue_num=1)
            nc.sync.dma_start(
                chunk1[1 + NT:1 + NT + NDUP, :].rearrange(
                    "(c p) w -> p c w", p=128),
                dupb[:, :, :])
        row1b = 1 + cfg.TA1 * 128 if cfg.TA1 else 0
        nc.gpsimd.collective_compute(
            "AllGather", Alu.bypass,
            replica_groups=[list(range(cfg.R))],
            ins=[chunk1[row1b:CHUNK, :].opt()],
            outs=[table1[:, :].rearrange("(r c) w -> r c w", r=cfg.R)[
                :, row1b:CHUNK, :].opt()])

        epool = ctx.enter_context(tc.tile_pool(name="edge", bufs=1))
        gpool = ctx.enter_context(tc.tile_pool(name="gpool", bufs=2))
        spool = ctx.enter_context(tc.tile_pool(name="spool", bufs=2))
        apool = ctx.enter_context(tc.tile_pool(name="apool", bufs=1))
        ypool = ctx.enter_context(tc.tile_pool(name="ypool", bufs=2))
        ppool = ctx.enter_context(tc.tile_pool(name="ppool", bufs=2))

        # ================= edge phase (per-layer schedule) ==================
        def edge_layer(layer, table, chunkx, pS, ROW, CH, NH, arl_t, out_cb,
                       groups, call_cols, colbase, SLOT_CAP, MSG_CAP):
            """layer: 1 or 2. CH: channels per head (32 / 64). NH: heads.
            arl_t: [128, T, NH] f32; pS: [128, T, NH] bf16 self-loop weights;
            chunkx: [CHUNK, ROW] own-rank projected rows (row 0 = sentinel).
            out_cb(t0, t1, unn, rec) per group.
            """
            HCL = CH * NH
            GMAX = max(t1 - t0 for (t0, t1, _, _) in groups)
            pair = table[:, :].rearrange("(n two) c -> n two c", two=2)
            lo_tab = pair[:, 0, :]      # even rows, stride 2*ROW
            hi_tab = pair[:, 1, :]      # odd rows, stride 2*ROW
            for gi, ((t0, t1, DL, DH), (lc0, lnc, hc0, hnc)) in enumerate(
                    zip(groups, call_cols)):
                G = t1 - t0
                SL, SH = G * DL, G * DH
                S = SL + SH
                g = gpool.tile([128, SLOT_CAP, ROW], BF16, tag="gbuf")
                nc.gpsimd.dma_gather(
                    g[:, 0:SL, :], lo_tab,
                    idx_s[:, colbase + lc0:colbase + lc0 + lnc],
                    num_idxs=SL * 128, num_idxs_reg=SL * 128,
                    elem_size=ROW, elem_step=2 * ROW, single_packet=False)
                nc.gpsimd.dma_gather(
                    g[:, SL:S, :], hi_tab,
                    idx_s[:, colbase + hc0:colbase + hc0 + hnc],
                    num_idxs=SH * 128, num_idxs_reg=SH * 128,
                    elem_size=ROW, elem_step=2 * ROW, single_packet=False,
                    queue_num=1)
                # own rows (self-loop h) for this group, plain dense DMA
                own = gpool.tile([128, GMAX, ROW], BF16, tag="own")
                nc.sync.dma_start(
                    own[:, 0:G, :],
                    chunkx[1 + t0 * 128:1 + t1 * 128, :].rearrange(
                        "(g p) c -> p g c", p=128))

                # pass A: logits + softmax numerators for both regions
                ps_r = []
                for ri, (off, D) in enumerate(((0, DL), (SL, DH))):
                    SD = G * D
                    gr = g[:, off:off + SD, :]
                    e = spool.tile([128, MSG_CAP, NH], F32, tag=f"e{ri}")
                    al = gr[:, :, HCL:HCL + 2 * NH].bitcast(F32)
                    e4 = e[:, 0:SD, :].rearrange("p (g d) h -> p g d h", g=G)
                    nc.vector.tensor_add(
                        e4, al.rearrange("p (g d) h -> p g d h", g=G),
                        _bc(arl_t[:, t0:t1, :].unsqueeze(2), (128, G, D, NH)))
                    nc.vector.scalar_tensor_tensor(
                        e[:, 0:SD, :], e[:, 0:SD, :], cfg.NEG, e[:, 0:SD, :],
                        op0=Alu.mult, op1=Alu.max)
                    p = spool.tile([128, MSG_CAP, NH], BF16, tag=f"p{ri}")
                    nc.scalar.activation(p[:, 0:SD, :], e[:, 0:SD, :], Act.Exp)
                    ps_r.append(p)

                # pass B: weight, aggregate
                parts, dens = [], []
                for ri, (off, D) in enumerate(((0, DL), (SL, DH))):
                    SD = G * D
                    gr = g[:, off:off + SD, :]
                    p = ps_r[ri]
                    den = spool.tile([128, GMAX, NH], F32, tag=f"den{ri}")
                    nc.vector.tensor_reduce(
                        den[:, 0:G, :],
                        p[:, 0:SD, :].rearrange("p (g d) h -> p g h d", g=G),
                        axis=mybir.AxisListType.X, op=Alu.add)
                    dens.append(den)
                    # msg <- p broadcast over c (doubling copies on the idle
                    # Scalar engine; last doubling absorbed into two
                    # half-multiplies on Vector), *= h
                    msg = ppool.tile([128, MSG_CAP, NH, CH], BF16, tag="msg")
                    nc.scalar.copy(msg[:, 0:SD, :, 0:1],
                                   p[:, 0:SD, :].unsqueeze(3))
                    half = CH // 2
                    k = 1
                    while k < half:
                        kk = min(k, half - k)
                        nc.scalar.copy(msg[:, 0:SD, :, k:k + kk],
                                       msg[:, 0:SD, :, 0:kk])
                        k += kk
                    gr4 = gr[:, :, 0:HCL].rearrange("p s (h c) -> p s h c",
                                                    h=NH)
                    msgh = msg[:, 0:SD, :, 0:half]
                    nc.vector.tensor_mul(msg[:, 0:SD, :, half:CH], msgh,
                                         gr4[:, :, :, half:CH])
                    nc.vector.tensor_mul(msgh, msgh, gr4[:, :, :, 0:half])
                    # tree-sum over slots within each tile -> [128, G, HCL]
                    msgt = msg[:, 0:SD, :, :].rearrange(
                        "p (g d) h c -> p g d (h c)", g=G)
                    part = apool.tile([128, GMAX, HCL], F32, tag=f"part{ri}")
                    cur = D
                    while cur > 2:
                        hh = cur // 2
                        nc.vector.tensor_add(
                            msgt[:, :, 0:hh, :], msgt[:, :, 0:hh, :],
                            msgt[:, :, cur - hh:cur, :])
                        cur -= hh
                    if cur == 2:
                        nc.vector.tensor_add(part[:, 0:G, :], msgt[:, :, 0, :],
                                             msgt[:, :, 1, :])
                    else:
                        nc.vector.tensor_copy(part[:, 0:G, :],
                                              msgt[:, :, 0, :])
                    parts.append(part)
                # self-loop contribution
                selfm = apool.tile([128, GMAX, NH, CH], F32, tag="selfm")
                nc.vector.tensor_mul(
                    selfm[:, 0:G, :, :],
                    own[:, 0:G, 0:HCL].rearrange("p g (h c) -> p g h c",
                                                 h=NH),
                    _bc(pS[:, t0:t1, :].unsqueeze(3), (128, G, NH, CH)))
                unn = apool.tile([128, GMAX, HCL], F32, tag="unn")
                nc.vector.tensor_add(unn[:, 0:G, :], parts[0][:, 0:G, :],
                                     parts[1][:, 0:G, :])
                nc.vector.tensor_add(
                    unn[:, 0:G, :], unn[:, 0:G, :],
                    selfm[:, 0:G, :, :].rearrange("p g h c -> p g (h c)"))
                den = spool.tile([128, GMAX, NH], F32, tag="dent")
                nc.vector.tensor_add(den[:, 0:G, :], dens[0][:, 0:G, :],
                                     dens[1][:, 0:G, :])
                nc.vector.tensor_add(den[:, 0:G, :], den[:, 0:G, :],
                                     pS[:, t0:t1, :])
                rec = spool.tile([128, GMAX, NH], F32, tag="rec")
                nc.vector.reciprocal(rec[:, 0:G, :], den[:, 0:G, :])
                out_cb(t0, t1, unn[:, 0:G, :], rec[:, 0:G, :])

        # ---- L1 epilogue: normalize, +b1, ELU, store h1 (per group);
        #      the L2 projection for the group's tiles runs inline so the
        #      first partial AG2 can fire while later L1 groups compute ----
        GMAX1 = max(t1 - t0 for (t0, t1, _, _) in sch.groups1)

        def l2_proj(t0, t1):
            for t in range(t0, t1):
                ps = psum.tile([128, OUT + 2], F32, tag="proj2")
                for k in range(KC):
                    nc.tensor.matmul(ps[:, :],
                                     h1T[:, k, t * 128:(t + 1) * 128],
                                     RHS2[:, k, :], start=(k == 0),
                                     stop=(k == KC - 1))
                nc.scalar.copy(tstage2[:, t, 0:OUT], ps[:, 0:OUT])
                al2_view = tstage2[:, t, OUT:OUT + 2].bitcast(F32)
                nc.vector.tensor_copy(al2_view[:, :], ps[:, OUT:OUT + 1])
                nc.vector.tensor_copy(ar2L[:, t, :], ps[:, OUT + 1:OUT + 2])
            nc.sync.dma_start(
                chunk2[1 + t0 * 128:1 + t1 * 128, :].rearrange(
                    "(t p) c -> p t c", p=128),
                tstage2[:, t0:t1, :])
            if sch.ta2 and t1 == sch.ta2:
                mA = 1 + sch.ta2 * 128
                nc.gpsimd.collective_compute(
                    "AllGather", Alu.bypass,
                    replica_groups=[list(range(cfg.R))],
                    ins=[chunk2[0:mA, :].opt()],
                    outs=[table2[0:cfg.R * mA, :].opt()])

        def l1_out(t0, t1, unn, rec):
            G = t1 - t0
            y = ypool.tile([128, GMAX1, H, HID], BF16, tag="y1")
            nc.vector.tensor_mul(
                y[:, 0:G, :, :],
                unn.rearrange("p g (h c) -> p g h c", h=H),
                _bc(rec.unsqueeze(3), (128, G, H, HID)))
            yf = y[:, 0:G, :, :].rearrange("p g h c -> p g (h c)")
            nc.vector.tensor_add(yf, yf, _bc(B1_s[:, :].unsqueeze(1),
                                             (128, G, HC1)))
            mn = ypool.tile([128, GMAX1, HC1], BF16, tag="mn1")
            nc.vector.tensor_scalar_min(mn[:, 0:G, :], yf, 0.0)
            nc.vector.tensor_scalar_max(yf, yf, 0.0)
            em = ypool.tile([128, GMAX1, HC1], BF16, tag="em1")
            nc.scalar.activation(em[:, 0:G, :], mn[:, 0:G, :], Act.Exp)
            h1t = ypool.tile([128, GMAX1, HC1], BF16, tag="h1t")
            nc.vector.scalar_tensor_tensor(h1t[:, 0:G, :], em[:, 0:G, :],
                                           -1.0, yf, op0=Alu.add, op1=Alu.add)
            nc.sync.dma_start(
                h1d[t0 * 128:t1 * 128, :].rearrange("(g p) c -> p g c", p=128),
                h1t[:, 0:G, :])
            for k in range(KC):
                nc.sync.dma_start_transpose(
                    h1T[:, k, t0 * 128:t1 * 128],
                    h1d[t0 * 128:t1 * 128, k * 128:(k + 1) * 128])
            l2_proj(t0, t1)

        h1T = epool.tile([128, KC, NT], BF16, tag="h1T")
        tstage2 = epool.tile([128, T, ROW2], BF16, tag="tstage2")
        nc.vector.memset(tstage2[:, :, :], 0.0)
        # sentinel row of chunk2 (covered by the first partial AG2)
        sent2e = epool.tile([1, ROW2], BF16, tag="sent2e")
        nc.vector.memset(sent2e[:, :], 0.0)
        nc.vector.memset(sent2e[:, OUT:OUT + 2].bitcast(F32), AL_SENT)
        nc.sync.dma_start(chunk2[0:1, :], sent2e[:, :])
        edge_layer(1, table1, chunk1, pS1, ROW1, HID, H, arL, l1_out,
                   sch.groups1, sch.call_cols1, 0, cfg.SLOT1, cfg.MSG1)

        eS2 = epool.tile([128, T, 1], F32, tag="eS2")
        al2L = tstage2[:, 0:T, OUT:OUT + 2].bitcast(F32)
        nc.vector.tensor_add(eS2[:, :, :], al2L, ar2L[:, :, :])
        nc.vector.scalar_tensor_tensor(
            eS2[:, :, :], eS2[:, :, :], cfg.NEG, eS2[:, :, :],
            op0=Alu.mult, op1=Alu.max)
        nc.scalar.activation(pS2[:, :, :], eS2[:, :, :], Act.Exp)
        dupb2 = epool.tile([128, NDUP // 128, ROW2], BF16, tag="dupb2")
        nc.gpsimd.dma_gather(
            dupb2[:, :, :], chunk2[0:1 + NT, :],
            dup_s[:, :],
            num_idxs=NDUP, num_idxs_reg=NDUP,
            elem_size=ROW2, single_packet=False, queue_num=1)
        nc.sync.dma_start(
            chunk2[1 + NT:1 + NT + NDUP, :].rearrange("(c p) w -> p c w", p=128),
            dupb2[:, :, :])
        # swap in the L2 gather-index section (overwrites L1's; the tile
        # framework orders this after the last L1 gather read)
        ncol2 = sch.offd - sch.off2
        nc.sync.dma_start(idx_s[:, 0:ncol2],
                          idxs_d[:, sch.off2:sch.off2 + ncol2])
        if sch.ta2:
            mA = 1 + sch.ta2 * 128
            mBp = CHUNK - mA + 1
            nc.gpsimd.collective_compute(
                "AllGather", Alu.bypass,
                replica_groups=[list(range(cfg.R))],
                ins=[chunk2[mA - 1:CHUNK, :].opt()],
                outs=[table2[cfg.R * mA:cfg.R * mA + cfg.R * mBp, :].opt()])
        else:
            nc.gpsimd.collective_compute(
                "AllGather", Alu.bypass,
                replica_groups=[list(range(cfg.R))],
                ins=[chunk2[0:CHUNK, :].opt()], outs=[table2[:, :].opt()])

        # ---- L2 epilogue: normalize, +b2, exp-sum; Ln deferred past loop ----
        ostage = epool.tile([128, T, OUT], F32, tag="ostage")
        ssumL = epool.tile([128, T, 1], F32, tag="ssumL")
        GMAX2 = max(t1 - t0 for (t0, t1, _, _) in sch.groups2)

        def l2_out(t0, t1, unn, rec):
            G = t1 - t0
            y = ypool.tile([128, GMAX2, OUT], F32, tag="y2")
            nc.vector.tensor_mul(y[:, 0:G, :], unn,
                                 _bc(rec, (128, G, OUT)))
            nc.vector.tensor_add(y[:, 0:G, :], y[:, 0:G, :],
                                 _bc(B2_s[:, :].unsqueeze(1), (128, G, OUT)))
            mx = spool.tile([128, GMAX2, 1], F32, tag="mx2")
            nc.vector.tensor_reduce(mx[:, 0:G, :], y[:, 0:G, :],
                                    axis=mybir.AxisListType.X, op=Alu.max)
            nc.vector.tensor_sub(ostage[:, t0:t1, :], y[:, 0:G, :],
                                 _bc(mx[:, 0:G, :], (128, G, OUT)))
            ex = spool.tile([128, GMAX2, OUT], F32, tag="ex2")
            nc.scalar.activation(ex[:, 0:G, :], ostage[:, t0:t1, :], Act.Exp)
            nc.vector.tensor_reduce(ssumL[:, t0:t1, :], ex[:, 0:G, :],
                                    axis=mybir.AxisListType.X, op=Alu.add)

        edge_layer(2, table2, chunk2, pS2, ROW2, OUT, 1, ar2L, l2_out,
                   sch.groups2, sch.call_cols2, 0, cfg.SLOT2, cfg.MSG2)
        lsL = epool.tile([128, T, 1], F32, tag="lsL")
        nc.scalar.activation(lsL[:, :, :], ssumL[:, :, :], Act.Ln)
        nc.vector.tensor_sub(ostage[:, :, :], ostage[:, :, :],
                             _bc(lsL[:, :, :], (128, T, OUT)))
        nc.sync.dma_start(out_d.ap().rearrange("(t p) c -> p t c", p=128),
                          ostage[:, :, :])

    nc.compile()
    return nc


def _host_inputs(cfg: Cfg, sch: Sched, inputs: dict):
    """Build per-rank in_maps from the full problem inputs."""
    x = np.asarray(inputs["x"], np.float32)
    W1 = np.asarray(inputs["W1"], np.float32)
    a1_src = np.asarray(inputs["a1_src"], np.float32)
    a1_dst = np.asarray(inputs["a1_dst"], np.float32)
    b1 = np.asarray(inputs["b1"], np.float32)
    W2 = np.asarray(inputs["W2"], np.float32)
    a2_src = np.asarray(inputs["a2_src"], np.float32)
    a2_dst = np.asarray(inputs["a2_dst"], np.float32)
    b2 = np.asarray(inputs["b2"], np.float32)
    H, HID, HC1, OUT = cfg.HEADS, cfg.HID, cfg.HC1, cfg.OUT

    # block-diagonal per-head attention matrices: al = h @ A1s
    A1s = np.zeros((HC1, H), np.float32)
    A1d = np.zeros((HC1, H), np.float32)
    for h in range(H):
        A1s[h * HID:(h + 1) * HID, h] = a1_src[h]
        A1d[h * HID:(h + 1) * HID, h] = a1_dst[h]

    common = {
        "W1": np.ascontiguousarray(W1),
        "W1T": np.ascontiguousarray(W1.T),
        "A1s": A1s, "A1d": A1d,
        "B1rep": np.tile(b1[None, :], (128, 1)).astype(np.float32),
        "W2": np.ascontiguousarray(W2),
        "W2T": np.ascontiguousarray(W2.T),
        "a2s": np.ascontiguousarray(a2_src.reshape(OUT, 1)),
        "a2d": np.ascontiguousarray(a2_dst.reshape(OUT, 1)),
        "B2rep": np.tile(b2[None, :], (128, 1)).astype(np.float32),
    }
    in_maps = []
    for r in range(cfg.R):
        m = dict(common)
        xp = np.zeros((cfg.NT, x.shape[1]), np.float32)
        xp[:cfg.NPR] = x[sch.perm[r]]
        m["xT"] = np.ascontiguousarray(xp.T)
        m["idxs"] = np.ascontiguousarray(sch.idx16[r])
        in_maps.append(m)
    return in_maps


def run(cfg: Cfg, inputs: dict, trace: bool = False, tmpdir: str | None = None):
    edge_index = np.asarray(inputs["edge_index"])
    # self-loops are handled densely in-kernel; only real edges are gathered
    src = edge_index[0].astype(np.int64)
    dst = edge_index[1].astype(np.int64)

    sch = build_schedule(cfg, src, dst)
    nc = build_program(cfg, sch)
    in_maps = _host_inputs(cfg, sch, inputs)
    res = bass_utils.run_bass_kernel_spmd(
        nc, in_maps, core_ids=list(range(cfg.R)), trace=trace, tmpdir=tmpdir)
    out = np.empty((cfg.N, cfg.OUT), np.float32)
    for r in range(cfg.R):
        o = res.results[r]["out"]
        out[sch.perm[r]] = o[:cfg.NPR]
    return out, res


def kernel(**inputs) -> np.ndarray:
    cfg = Cfg()
    out, _ = run(cfg, inputs)
    return out


if __name__ == "__main__":
    import reference
    inputs = {k: np.asarray(v) for k, v in reference.setup_inputs().items()}
    out = kernel(**inputs)
    exp = np.asarray(reference.reference(**reference.setup_inputs()))
    err = np.abs(out - exp).max() / (np.abs(exp).max() + 1e-12)
    print("rel err:", err)


# revision 26
# speedup vs baseline: 1.0008x; 1.0008x over previous
"""2-layer GAT (gnn_message_passing) on 8 TRN2 NeuronCores.

Strategy (graph/data parallel, per sharding hint):
  - Nodes are partitioned across 8 ranks (6250 dst nodes each). Each rank owns
    the segment-softmax + aggregation for its destination nodes.
  - Per layer, every rank computes the projected features (h = x @ W,
    attention source/dest logits al/ar fused into the same matmul via an
    augmented RHS) for ITS OWN nodes, writes them as rows of a gather table
    (768B rows for layer 1: 256 bf16 h + 8 f32 al; 256B rows for layer 2),
    then an AllGather replicates the full table to every rank.
  - Edge stage: destinations are degree-sorted and packed into tiles of 128
    (dst on partitions); consecutive tiles are grouped with a UNIFORM padded
    slot count per group so that the whole group is one rectangular grid
    [128, G, D, ...] and every vector/scalar op covers the full group in a
    single instruction. Source rows are fetched with dma_gather (SWDGE
    indexed gather). Since gather indices are int16 (max 32767) and the
    table has ~60k rows, rows are addressed through an even/odd pair view
    (idx = row//2): each group issues one even-window and one odd-window
    gather, so every edge needs a parity side.
  - Slot-padding control: per-tile slot counts are max(c_even), max(c_odd)
    over the (degree-sorted) tile's dsts. A host-side balancer chooses each
    source node's row parity to balance every dst's in-edge parity split;
    residual "blocker" sources are DUPLICATED (their row is copied once more
    at the opposite parity at the end of the owner rank's chunk), making all
    their edges per-edge flexible. This brings padded slots within ~2% of
    the true floor (max in-degree per tile). L1 and L2 use separate group
    schedules: L2 rows are 4x smaller so its groups merge ~3x more tiles.
  - Segment softmax is all free-dim math: e = leakyrelu(al_src + ar_dst) on
    the slot grid, p = exp(e) (no max-subtract needed at these magnitudes;
    mathematically identical), denom = free-dim reduce, normalization applied
    AFTER aggregation (divide the aggregated sums by denom).
  - Aggregation: msg = p (broadcast over channels by doubling copies on the
    otherwise-idle Scalar engine) * h_src, then a pairwise tree of wide
    tensor adds along the slot dim.
  - Padding slots read a sentinel table row (h = 0, al = -1e30 -> p = 0).

The full output is assembled on the host from the 8 per-rank outputs
(undoing the degree-sort permutation).
"""

import sys
from contextlib import ExitStack
from dataclasses import dataclass

import numpy as np

for _p in ("/opt/trn_rl_repo",):
    if _p not in sys.path:
        sys.path.insert(0, _p)

import concourse.bass as bass
import concourse.bacc as bacc
import concourse.mybir as mybir
import concourse.tile as tile
from concourse import bass_utils


F32 = mybir.dt.float32
BF16 = mybir.dt.bfloat16
I16 = mybir.dt.int16
AL_SENT = -1.0e30
Alu = mybir.AluOpType
Act = mybir.ActivationFunctionType


@dataclass
class Cfg:
    N: int = 50000
    E: int = 500000          # edges before self-loops
    F_IN: int = 128
    HID: int = 32
    HEADS: int = 8
    OUT: int = 64
    NEG: float = 0.2
    R: int = 8
    SLOT1: int = 36          # L1 max uniform slots per gather group
    MSG1: int = 28           # L1 max slots per region
    SLOT2: int = 104         # L2 caps (rows 4x smaller; pools shared with L1)
    MSG2: int = 96
    NDUP: int = 1664         # duplicate rows per rank (multiple of 128, even)
    TA1: int = 0             # AG1 split tile boundary (0 = no split)
    TA2: int = 37            # AG2 split tile boundary (0 = no split)

    @property
    def HC1(self):
        return self.HEADS * self.HID     # 256

    @property
    def NPR(self):
        return self.N // self.R

    @property
    def T(self):
        return (self.NPR + 127) // 128   # dst tiles per rank

    @property
    def NT(self):
        return self.T * 128

    @property
    def CHUNK(self):
        # sentinel + staged rows (NT >= NPR) + duplicate rows; must be odd
        return 1 + self.NT + self.NDUP

    @property
    def TROWS(self):
        return self.R * self.CHUNK

    @property
    def ROW1(self):
        return 384                       # bf16 elems: 256 h + 16 (8xf32 al) + pad

    @property
    def ROW2(self):
        return 128                       # bf16 elems: 64 h2 + 2 (1xf32 al2) + pad


@dataclass
class Sched:
    perm: np.ndarray          # [R, NPR] perm[r][pos] = global node id
    groups1: list             # L1 groups: (t0, t1, DL, DH)
    call_cols1: list          # per group, section-relative (lo0, lnc, hi0, hnc)
    groups2: list             # L2 groups
    call_cols2: list
    idx16: np.ndarray         # [R, 128, TOTCOL] int16 (L1 | L2 | dup)
    off2: int                 # column offset of the L2 section
    offd: int                 # column offset of the dup section
    ta2: int = 0              # AG2 split tile boundary (0 = no split)
    sent_hi2: int = 0         # odd-pad sentinel idx for the L2 table


def _pack_idx(vals: np.ndarray) -> np.ndarray:
    """int32 row-idx values -> the [128, n/16] int16 SWDGE index layout."""
    assert vals.min() >= 0 and vals.max() < 32768, (vals.min(), vals.max())
    return np.tile(vals.astype(np.int16).reshape(-1, 16).T, (8, 1))


def build_schedule(cfg: Cfg, src: np.ndarray, dst: np.ndarray) -> Sched:
    N, R, NPR, T = cfg.N, cfg.R, cfg.NPR, cfg.T
    CHUNK, NT = cfg.CHUNK, cfg.NT
    assert CHUNK % 2 == 1 and 4 * CHUNK < 32768
    deg = np.bincount(dst, minlength=N).astype(np.int64)

    # ---- global degree-sorted tiles (1024 nodes per global tile) ----
    gorder = np.argsort(-deg, kind="stable")
    gtile = np.empty(N, np.int64)
    for t in range(T):
        gtile[gorder[t * 1024:(t + 1) * 1024]] = t
    tile_of_dst = gtile[dst]
    maxdeg_t = np.array([max(1, deg[gorder[t * 1024:(t + 1) * 1024]].max())
                         for t in range(T)])

    eorder = np.argsort(src, kind="stable")
    s_sorted = src[eorder]
    d_sorted = dst[eorder]
    starts = np.searchsorted(s_sorted, np.arange(N + 1))

    # ---- parity balancing: conflict-free vectorized greedy ----
    rng = np.random.default_rng(12345)
    parity = np.zeros(N, np.int8)
    tile_nodes = []
    for t in range(T):
        nodes = gorder[t * 1024:(t + 1) * 1024]
        tile_nodes.append(nodes)
        p = np.zeros(len(nodes), np.int8)
        p[:len(nodes) // 2] = 1
        rng.shuffle(p)
        parity[nodes] = p

    c_e = np.zeros(N, np.int32)
    c_o = np.zeros(N, np.int32)
    pe = parity[src]
    np.add.at(c_e, dst[pe == 0], 1)
    np.add.at(c_o, dst[pe == 1], 1)

    # alternate the ceil side per tile so parity peaks (and hence dup-copy
    # parity demand) split ~evenly between the even and odd windows
    ceil_half = np.ceil((maxdeg_t + 1) / 2).astype(np.int64)
    Te = np.where(np.arange(T) % 2 == 0, ceil_half, (maxdeg_t + 1) - ceil_half)
    To = (maxdeg_t + 1) - Te
    TeD = Te[tile_of_dst]
    ToD = To[tile_of_dst]
    imb = np.zeros(T, np.int64)
    CAP, W = 12, 8.0

    def pen(c, Tt):
        return np.where(c > Tt, W ** np.minimum(c - Tt, 6), 0.0)

    for rnd in range(120):
        ceD = c_e[dst]
        coD = c_o[dst]
        d_eo = (pen(ceD - 1, TeD) - pen(ceD, TeD)) + (pen(coD + 1, ToD) - pen(coD, ToD))
        d_oe = (pen(coD - 1, ToD) - pen(coD, ToD)) + (pen(ceD + 1, TeD) - pen(ceD, TeD))
        cum_eo = np.concatenate([[0.], np.cumsum(d_eo[eorder])])
        cum_oe = np.concatenate([[0.], np.cumsum(d_oe[eorder])])
        g_eo = -(cum_eo[starts[1:]] - cum_eo[starts[:-1]])
        g_oe = -(cum_oe[starts[1:]] - cum_oe[starts[:-1]])
        gain = np.where(parity == 0, g_eo, g_oe)
        cand = np.where(gain > 1e-9)[0]
        if len(cand) == 0:
            break
        cand = cand[np.argsort(-gain[cand])]
        dirty = np.zeros(N, bool)
        napp = 0
        for u in cand:
            ds = d_sorted[starts[u]:starts[u + 1]]
            if dirty[ds].any():
                continue
            t = gtile[u]
            delta = 1 if parity[u] == 0 else -1
            if abs(imb[t] + delta) > CAP:
                continue
            dirty[ds] = True
            imb[t] += delta
            napp += 1
            if parity[u] == 0:
                c_e[ds] -= 1
                c_o[ds] += 1
                parity[u] = 1
            else:
                c_o[ds] -= 1
                c_e[ds] += 1
                parity[u] = 0
        if napp == 0:
            break
    # repair per-tile parity balance to exact 50/50
    for t in range(T):
        while imb[t] != 0:
            nodes = tile_nodes[t]
            want = 1 if imb[t] > 0 else 0
            pool = nodes[parity[nodes] == want]
            ceD = c_e[dst]
            coD = c_o[dst]
            if want == 1:
                dpe = (pen(coD - 1, ToD) - pen(coD, ToD)) + (pen(ceD + 1, TeD) - pen(ceD, TeD))
            else:
                dpe = (pen(ceD - 1, TeD) - pen(ceD, TeD)) + (pen(coD + 1, ToD) - pen(coD, ToD))
            cum = np.concatenate([[0.], np.cumsum(dpe[eorder])])
            gg = -(cum[starts[1:]] - cum[starts[:-1]])
            bu = pool[np.argmax(gg[pool])]
            ds = d_sorted[starts[bu]:starts[bu + 1]]
            if want == 1:
                c_o[ds] -= 1
                c_e[ds] += 1
                parity[bu] = 0
                imb[t] -= 1
            else:
                c_e[ds] -= 1
                c_o[ds] += 1
                parity[bu] = 1
                imb[t] += 1

    # ---- duplicate "blocker" sources until forced maxima reach the floor ----
    dup = np.zeros(N, bool)
    max_dups = (cfg.NDUP // 2 - 32) * 2 * R  # conservative global budget

    def forced_stats():
        f_e = np.zeros(N, np.int32)
        f_o = np.zeros(N, np.int32)
        m = ~dup[src]
        pp = parity[src]
        np.add.at(f_e, dst[m & (pp == 0)], 1)
        np.add.at(f_o, dst[m & (pp == 1)], 1)
        FE = np.zeros(T, np.int64)
        FO = np.zeros(T, np.int64)
        np.maximum.at(FE, tile_of_dst, f_e[dst])
        np.maximum.at(FO, tile_of_dst, f_o[dst])
        return f_e, f_o, np.maximum(FE, 1), np.maximum(FO, 1)

    for it in range(200):
        f_e, f_o, FE, FO = forced_stats()
        bind = (FE + FO) > maxdeg_t
        if not bind.any() or dup.sum() >= max_dups:
            break
        peak_e = (bind[tile_of_dst] & (f_e[dst] == FE[tile_of_dst])
                  & (parity[src] == 0) & ~dup[src])
        peak_o = (bind[tile_of_dst] & (f_o[dst] == FO[tile_of_dst])
                  & (parity[src] == 1) & ~dup[src])
        sc = np.zeros(N, np.int64)
        np.add.at(sc, src[peak_e | peak_o], 1)
        order = np.argsort(-sc)
        take = order[sc[order] > 0][:200]
        if len(take) == 0:
            break
        dup[take] = True
    f_e, f_o, FE, FO = forced_stats()

    # ---- per-tile slot budgets + flexible (dup-sourced) edge assignment ----
    B_t = np.maximum(FE + FO, maxdeg_t)
    mid = np.where(np.arange(T) % 2 == 0, np.ceil(B_t / 2),
                   np.floor(B_t / 2)).astype(np.int64)
    Te_t = np.clip(mid, FE, B_t - FO)
    flex_cnt = np.zeros(N, np.int32)
    np.add.at(flex_cnt, dst[dup[src]], 1)
    TeN = Te_t[gtile]                        # per-dst lo budget
    lo_cnt = f_e + np.minimum(flex_cnt, np.maximum(0, TeN - f_e)).astype(np.int32)
    assert (lo_cnt <= TeN).all()
    assert ((deg - lo_cnt) <= (B_t - Te_t)[gtile]).all()

    # ---- placement: assign nodes to (rank, position) honoring parity ----
    perm = np.empty((R, NPR), np.int64)
    rank_of = np.empty(N, np.int64)
    sortpos = np.empty(N, np.int64)
    for t in range(T):
        nodes = tile_nodes[t]
        k = len(nodes) // R
        # dup'd nodes first within each parity class, then stride-8 interleave
        # across ranks so each rank gets an equal share of dup copies
        ev_nodes = nodes[parity[nodes] == 0]
        od_nodes = nodes[parity[nodes] == 1]
        ev_nodes = ev_nodes[np.argsort(~dup[ev_nodes], kind="stable")]
        od_nodes = od_nodes[np.argsort(~dup[od_nodes], kind="stable")]
        for r in range(R):
            pos = t * 128 + np.arange(k)
            pp = (r + 1 + pos) % 2            # row parity of each position
            ev_pos = pos[pp == 0]
            od_pos = pos[pp == 1]
            a = ev_nodes[r::R]
            b = od_nodes[r::R]
            assert len(a) == len(ev_pos) and len(b) == len(od_pos), (t, r)
            perm[r, ev_pos] = a
            perm[r, od_pos] = b
            rank_of[a] = r
            rank_of[b] = r
            sortpos[a] = ev_pos
            sortpos[b] = od_pos

    row_of = rank_of * CHUNK + 1 + sortpos
    assert ((row_of % 2) == parity).all()

    # ---- dup row placement: copy at opposite parity in owner rank chunk ----
    dup_row = np.full(N, -1, np.int64)
    dup_vals = np.zeros((R, cfg.NDUP), np.int64)
    for r in range(R):
        dn = np.where(dup & (rank_of == r))[0]
        need_even = dn[parity[dn] == 1]
        need_odd = dn[parity[dn] == 0]
        slots = np.arange(cfg.NDUP)
        spar = (r + 1 + NT + slots) % 2
        ev_slots = slots[spar == 0]
        od_slots = slots[spar == 1]
        assert len(need_even) <= len(ev_slots) and len(need_odd) <= len(od_slots), (
            r, len(need_even), len(need_odd))
        for nn, ss in ((need_even, ev_slots), (need_odd, od_slots)):
            rows = r * CHUNK + 1 + NT + ss[:len(nn)]
            dup_row[nn] = rows
            dup_vals[r, ss[:len(nn)]] = 1 + sortpos[nn]
    row_even = np.where(parity == 0, row_of, dup_row)
    row_odd = np.where(parity == 1, row_of, dup_row)

    # ---- per-edge side + slot assignment ----
    dkey = rank_of[dst] * NPR + sortpos[dst]
    side = np.where(dup[src], -1, parity[src]).astype(np.int64)  # -1 = flex
    order0 = np.lexsort((side, dkey))
    s_side = side[order0]
    s_dkey = dkey[order0]
    cnt = np.bincount(dkey, minlength=R * NPR)
    start = np.concatenate([[0], np.cumsum(cnt)])[:-1]
    pos_in = np.arange(len(order0)) - start[s_dkey]
    nflo = np.bincount(dkey[side == 0], minlength=R * NPR)
    lo_cnt_d = lo_cnt[dst[order0]]
    flex_lo_quota = lo_cnt_d - nflo[s_dkey]
    is_flex = s_side == -1
    new_side = np.where(is_flex, np.where(pos_in < flex_lo_quota, 0, 1), s_side)

    order1 = np.lexsort((new_side, s_dkey))
    f_side = new_side[order1]
    f_dkey = s_dkey[order1]
    f_edge = order0[order1]
    pos_f = np.arange(len(order1)) - start[f_dkey]
    nlo_f = np.bincount(f_dkey[f_side == 0], minlength=R * NPR)
    slot = np.where(f_side == 0, pos_f, pos_f - nlo_f[f_dkey])
    f_src = src[f_edge]
    f_row = np.where(f_side == 0, row_even[f_src], row_odd[f_src])
    assert (f_row >= 0).all()
    assert ((f_row % 2) == f_side).all()

    # per-tile slot maxima (shared by all ranks)
    D_lo = np.zeros(T, np.int64)
    D_hi = np.zeros(T, np.int64)
    nlo_g = nlo_f.reshape(R, NPR)
    nhi_g = (cnt - nlo_f).reshape(R, NPR)
    for t in range(T):
        sl = slice(t * 128, min((t + 1) * 128, NPR))
        D_lo[t] = max(1, nlo_g[:, sl].max())
        D_hi[t] = max(1, nhi_g[:, sl].max())

    def make_groups(SLOT_CAP, MSG_CAP):
        groups = []
        t0 = 0
        while t0 < T:
            t1, DL, DH = t0, 0, 0
            while t1 < T:
                nDL = max(DL, int(D_lo[t1]))
                nDH = max(DH, int(D_hi[t1]))
                G1 = t1 - t0 + 1
                if (G1 * (nDL + nDH) > SLOT_CAP
                        or G1 * max(nDL, nDH) > MSG_CAP):
                    break
                DL, DH, t1 = nDL, nDH, t1 + 1
            assert t1 > t0, (t0, D_lo[t0], D_hi[t0])
            groups.append((t0, t1, DL, DH))
            t0 = t1
        return groups

    def layout(groups):
        """Section-relative column layout. Returns (call_cols, pos_base, ncols)."""
        call_cols = []
        pos_base = np.zeros((T, 2), np.int64)
        col = 0
        for (t0, t1, DL, DH) in groups:
            G = t1 - t0
            lo0 = col
            for g, t in enumerate(range(t0, t1)):
                pos_base[t, 0] = col * 16 + g * DL * 128
            col += G * DL * 8
            hi0 = col
            for g, t in enumerate(range(t0, t1)):
                pos_base[t, 1] = col * 16 + g * DH * 128
            col += G * DH * 8
            call_cols.append((lo0, G * DL * 8, hi0, G * DH * 8))
        return call_cols, pos_base, col

    groups1 = make_groups(cfg.SLOT1, cfg.MSG1)
    groups2 = make_groups(cfg.SLOT2, cfg.MSG2)

    # ---- L2 table row mapping (two-piece layout for the split AllGather) ----
    # piece A = all ranks' chunk rows [0, mA); piece B = chunk rows
    # [mA-1, CHUNK) per rank (leading duplicate of row mA-1 keeps mB' odd so
    # that row parity matches the chunk parity used for side assignment).
    if cfg.TA2:
        ta2 = 0
        for (g_t0, g_t1, _dl, _dh) in groups1:
            if g_t1 >= cfg.TA2:
                ta2 = g_t1
                break
        mA = 1 + ta2 * 128
        mBp = CHUNK - mA + 1
        assert mA % 2 == 1 and mBp % 2 == 1

        def row2_of(c, r):
            return np.where(c < mA, r * mA + c,
                            R * mA + r * mBp + c - mA + 1)
        sent_hi2 = mA // 2  # rank-1 sentinel: row2 = mA (odd)
    else:
        ta2 = 0

        def row2_of(c, r):
            return r * CHUNK + c
        sent_hi2 = (CHUNK - 1) // 2
    chunk_row = row_of - rank_of * CHUNK
    row2_node = row2_of(chunk_row, rank_of)
    dup_chunk_row = np.where(dup_row >= 0, dup_row % CHUNK, 0)
    dup_row2 = np.where(dup_row >= 0,
                        row2_of(dup_chunk_row, rank_of), -1)
    assert ((row2_node % 2) == parity).all()
    assert ((dup_row2[dup] % 2) == (1 - parity[dup])).all()
    row_even2 = np.where(parity == 0, row2_node, dup_row2)
    row_odd2 = np.where(parity == 1, row2_node, dup_row2)
    f_row2 = np.where(f_side == 0, row_even2[f_src], row_odd2[f_src])
    assert (f_row2 >= 0).all() and ((f_row2 % 2) == f_side).all()
    call_cols1, pos_base1, ncol1 = layout(groups1)
    call_cols2, pos_base2, ncol2 = layout(groups2)
    ndup_cols = cfg.NDUP // 16
    TOTCOL = ncol1 + ncol2 + ndup_cols

    SENT_LO = 0                 # even pad: rank-0 sentinel row 0
    SENT_HI = (CHUNK - 1) // 2  # odd pad: rank-1 sentinel row CHUNK (odd)

    e_rank = f_dkey // NPR
    e_pos = f_dkey % NPR
    e_tile = e_pos // 128
    e_part = e_pos % 128
    idx16 = np.empty((R, 128, TOTCOL), np.int16)
    for r in range(R):
        m = (e_rank == r)
        mlo = m & (f_side == 0)
        mhi = m & (f_side == 1)
        sections = []
        for (groups, call_cols, pos_base, ncol, rows, s_hi) in (
                (groups1, call_cols1, pos_base1, ncol1, f_row, SENT_HI),
                (groups2, call_cols2, pos_base2, ncol2, f_row2, sent_hi2)):
            vals = np.empty(ncol * 16, np.int32)
            for (lc0, lnc, hc0, hnc) in call_cols:
                vals[lc0 * 16:(lc0 + lnc) * 16] = SENT_LO
                vals[hc0 * 16:(hc0 + hnc) * 16] = s_hi
            p_lo = pos_base[e_tile[mlo], 0] + slot[mlo] * 128 + e_part[mlo]
            vals[p_lo] = rows[mlo] // 2
            p_hi = pos_base[e_tile[mhi], 1] + slot[mhi] * 128 + e_part[mhi]
            vals[p_hi] = (rows[mhi] - 1) // 2
            sections.append(_pack_idx(vals))
        sections.append(_pack_idx(dup_vals[r]))
        idx16[r] = np.concatenate(sections, axis=1)

    return Sched(perm=perm, groups1=groups1, call_cols1=call_cols1,
                 groups2=groups2, call_cols2=call_cols2, idx16=idx16,
                 off2=ncol1, offd=ncol1 + ncol2, ta2=ta2, sent_hi2=sent_hi2)


def _bc(ap, shape):
    """broadcast an AP to shape (step-0 dims)"""
    return ap.broadcast_to(list(shape))


def build_program(cfg: Cfg, sch: Sched):
    """Build the single SPMD Bass program. Returns nc."""
    nc = bacc.Bacc("TRN2", target_bir_lowering=False, debug=False,
                   num_devices=cfg.R, num_swdge_queues=2)
    T, NPR, CHUNK, TROWS, NT = cfg.T, cfg.NPR, cfg.CHUNK, cfg.TROWS, cfg.NT
    HC1, H, HID, OUT = cfg.HC1, cfg.HEADS, cfg.HID, cfg.OUT
    ROW1, ROW2, NDUP = cfg.ROW1, cfg.ROW2, cfg.NDUP
    TOTCOL = sch.idx16.shape[2]

    # ---- I/O ----
    xT = nc.dram_tensor("xT", [cfg.F_IN, NT], F32, kind="ExternalInput")
    idxs_d = nc.dram_tensor("idxs", [128, TOTCOL], I16, kind="ExternalInput")
    W1_d = nc.dram_tensor("W1", [cfg.F_IN, HC1], F32, kind="ExternalInput")
    W1T_d = nc.dram_tensor("W1T", [HC1, cfg.F_IN], F32, kind="ExternalInput")
    A1s_d = nc.dram_tensor("A1s", [HC1, H], F32, kind="ExternalInput")
    A1d_d = nc.dram_tensor("A1d", [HC1, H], F32, kind="ExternalInput")
    B1_d = nc.dram_tensor("B1rep", [128, HC1], F32, kind="ExternalInput")
    W2_d = nc.dram_tensor("W2", [HC1, OUT], F32, kind="ExternalInput")
    W2T_d = nc.dram_tensor("W2T", [OUT, HC1], F32, kind="ExternalInput")
    a2s_d = nc.dram_tensor("a2s", [OUT, 1], F32, kind="ExternalInput")
    a2d_d = nc.dram_tensor("a2d", [OUT, 1], F32, kind="ExternalInput")
    B2_d = nc.dram_tensor("B2rep", [128, OUT], F32, kind="ExternalInput")
    out_d = nc.dram_tensor("out", [NT, OUT], F32, kind="ExternalOutput")

    KC = HC1 // 128   # contraction chunks over HC1 (2)

    with tile.TileContext(nc) as tc, ExitStack() as ctx:
        dram = ctx.enter_context(tc.tile_pool(name="dram", bufs=1, space="DRAM"))
        const = ctx.enter_context(tc.tile_pool(name="const", bufs=1))
        psum = ctx.enter_context(tc.tile_pool(name="psum", bufs=2, space="PSUM"))

        # DRAM scratch
        chunk1 = dram.tile([CHUNK, ROW1], BF16)
        table1 = dram.tile([TROWS, ROW1], BF16, addr_space="Shared")
        chunk2 = dram.tile([CHUNK, ROW2], BF16)
        # table2 is NOT in the Shared address space: Shared DRAM allows a
        # single writer instruction, and the AG2 overlap needs two partial
        # AllGathers writing disjoint (contiguous) row ranges of the
        # two-piece layout (one extra row per rank in piece B)
        TROWS2 = cfg.R * (CHUNK + 1) if sch.ta2 else TROWS
        table2 = dram.tile([TROWS2, ROW2], BF16,
                           addr_space="Shared" if not sch.ta2 else "Local")
        h1d = dram.tile([NT, HC1], BF16)

        # ---- persistent constants ----
        # idx SBUF is swapped per layer: [active-layer cols | dup cols]
        MAXCOL = max(sch.off2, sch.offd - sch.off2)
        idx_s = const.tile([128, MAXCOL], I16, tag="idx")
        nc.sync.dma_start(idx_s[:, 0:sch.off2], idxs_d[:, 0:sch.off2])
        dup_s = const.tile([128, NDUP // 16], I16, tag="idxdup")
        nc.sync.dma_start(dup_s[:, :], idxs_d[:, sch.offd:sch.offd + NDUP // 16])
        RHS1 = const.tile([128, HC1 + 2 * H], F32, tag="rhs1")
        nc.sync.dma_start(RHS1[:, 0:HC1], W1_d[:, :])
        B1_s = const.tile([128, HC1], BF16, tag="b1")
        B1f_s = const.tile([128, HC1], F32, tag="b1f")
        nc.sync.dma_start(B1f_s[:, :], B1_d[:, :])
        nc.vector.tensor_copy(B1_s[:, :], B1f_s[:, :])
        B2_s = const.tile([128, OUT], F32, tag="b2")
        nc.sync.dma_start(B2_s[:, :], B2_d[:, :])
        arL = const.tile([128, T, H], F32, tag="arL")
        nc.vector.memset(arL[:, :, :], 0.0)
        ar2L = const.tile([128, T, 1], F32, tag="ar2L")
        nc.vector.memset(ar2L[:, :, :], 0.0)
        RHS2 = const.tile([128, KC, OUT + 2], BF16, tag="rhs2")
        nc.gpsimd.dma_start(RHS2[:, :, 0:OUT],
                            W2_d.ap().rearrange("(k p) c -> p k c", p=128))
        pS1 = const.tile([128, T, H], BF16, tag="pS1")
        pS2 = const.tile([128, T, 1], BF16, tag="pS2")

        # ================= phase 1: projection + table 1 ====================
        with tc.tile_pool(name="ph1", bufs=1) as ph1:
            xT_s = ph1.tile([128, NT], F32, tag="xT")
            nc.sync.dma_start(xT_s[:, :], xT[:, :])
            W1T_s = ph1.tile([128, KC, 128], F32, tag="w1t")
            nc.sync.dma_start(W1T_s[:, :, :],
                              W1T_d.ap().rearrange("(k p) f -> p k f", p=128))
            A1s_s = ph1.tile([128, KC, H], F32, tag="a1s")
            nc.sync.dma_start(A1s_s[:, :, :],
                              A1s_d.ap().rearrange("(k p) h -> p k h", p=128))
            A1d_s = ph1.tile([128, KC, H], F32, tag="a1d")
            nc.sync.dma_start(A1d_s[:, :, :],
                              A1d_d.ap().rearrange("(k p) h -> p k h", p=128))
            W2T_s = ph1.tile([OUT, HC1], F32, tag="w2t")
            nc.sync.dma_start(W2T_s[:, :], W2T_d[:, :])
            a2s_s = ph1.tile([OUT, 1], F32, tag="a2s")
            nc.sync.dma_start(a2s_s[:, :], a2s_d[:, :])
            a2d_s = ph1.tile([OUT, 1], F32, tag="a2d")
            nc.sync.dma_start(a2d_s[:, :], a2d_d[:, :])

            # fold attention vectors into projection RHS
            for (dst_off, A_s) in ((HC1, A1s_s), (HC1 + H, A1d_s)):
                ps = psum.tile([128, H], F32, tag="wprep")
                for k in range(KC):
                    nc.tensor.matmul(ps[:, :], W1T_s[:, k, :], A_s[:, k, :],
                                     start=(k == 0), stop=(k == KC - 1))
                nc.vector.tensor_copy(RHS1[:, dst_off:dst_off + H], ps[:, :])
            for (dst_off, a_s) in ((OUT, a2s_s), (OUT + 1, a2d_s)):
                for k in range(KC):
                    ps = psum.tile([128, 1], F32, tag="wprep2")
                    nc.tensor.matmul(ps[:, :], W2T_s[:, k * 128:(k + 1) * 128],
                                     a_s[:, :], start=True, stop=True)
                    nc.vector.tensor_copy(RHS2[:, k, dst_off:dst_off + 1],
                                          ps[:, :])

            # sentinel row -> chunk row 0 (h = 0, al = -1e30); written first
            # so the first partial AllGather can cover it
            sent1 = ph1.tile([1, ROW1], BF16, tag="sent1")
            nc.vector.memset(sent1[:, :], 0.0)
            nc.vector.memset(sent1[:, HC1:HC1 + 2 * H].bitcast(F32), AL_SENT)
            nc.sync.dma_start(chunk1[0:1, :], sent1[:, :])
            tstage = ph1.tile([128, T, ROW1], BF16, tag="tstage1")
            nc.vector.memset(tstage[:, :, :], 0.0)
            TA1 = cfg.TA1
            for t in range(T):
                ps = psum.tile([128, HC1 + 2 * H], F32, tag="proj1")
                nc.tensor.matmul(ps[:, :], xT_s[:, t * 128:(t + 1) * 128],
                                 RHS1[:, :], start=True, stop=True)
                nc.scalar.copy(tstage[:, t, 0:HC1], ps[:, 0:HC1])
                al_view = tstage[:, t, HC1:HC1 + 2 * H].bitcast(F32)
                nc.vector.tensor_copy(al_view[:, :], ps[:, HC1:HC1 + H])
                nc.vector.tensor_copy(arL[:, t, :],
                                      ps[:, HC1 + H:HC1 + 2 * H])
                if TA1 and t == TA1 - 1:
                    nc.sync.dma_start(
                        chunk1[1:1 + TA1 * 128, :].rearrange(
                            "(t p) c -> p t c", p=128),
                        tstage[:, 0:TA1, :])
                    nc.gpsimd.collective_compute(
                        "AllGather", Alu.bypass,
                        replica_groups=[list(range(cfg.R))],
                        ins=[chunk1[0:1 + TA1 * 128, :].opt()],
                        outs=[table1[:, :].rearrange(
                            "(r c) w -> r c w", r=cfg.R)[
                            :, 0:1 + TA1 * 128, :].opt()])
            # dense self-loop weights: pS1 = exp(leakyrelu(al + ar))
            eS = ph1.tile([128, T, H], F32, tag="eS")
            alL = tstage[:, 0:T, HC1:HC1 + 2 * H].bitcast(F32)
            nc.vector.tensor_add(eS[:, :, :], alL, arL[:, :, :])

            nc.vector.scalar_tensor_tensor(
                eS[:, :, :], eS[:, :, :], cfg.NEG, eS[:, :, :],
                op0=Alu.mult, op1=Alu.max)
            nc.scalar.activation(pS1[:, :, :], eS[:, :, :], Act.Exp)
            TA1 = cfg.TA1
            nc.sync.dma_start(
                chunk1[1 + TA1 * 128:1 + NT, :].rearrange(
                    "(t p) c -> p t c", p=128),
                tstage[:, TA1:T, :])
            # duplicate rows: indexed re-fetch of own chunk rows, append
            dupb = ph1.tile([128, NDUP // 128, ROW1], BF16, tag="dupb")
            nc.gpsimd.dma_gather(
                dupb[:, :, :], chunk1[0:1 + NT, :],
                dup_s[:, :],
                num_idxs=NDUP, num_idxs_reg=NDUP,
                elem_size=ROW1, single_packet=False, queue_num=1)
            nc.sync.dma_start(
                chunk1[1 + NT:1 + NT + NDUP, :].rearrange(
                    "(c p) w -> p c w", p=128),
                dupb[:, :, :])
        row1b = 1 + cfg.TA1 * 128 if cfg.TA1 else 0
        nc.gpsimd.collective_compute(
            "AllGather", Alu.bypass,
            replica_groups=[list(range(cfg.R))],
            ins=[chunk1[row1b:CHUNK, :].opt()],
            outs=[table1[:, :].rearrange("(r c) w -> r c w", r=cfg.R)[
                :, row1b:CHUNK, :].opt()])

        epool = ctx.enter_context(tc.tile_pool(name="edge", bufs=1))
        gpool = ctx.enter_context(tc.tile_pool(name="gpool", bufs=2))
        spool = ctx.enter_context(tc.tile_pool(name="spool", bufs=2))
        apool = ctx.enter_context(tc.tile_pool(name="apool", bufs=1))
        ypool = ctx.enter_context(tc.tile_pool(name="ypool", bufs=2))
        ppool = ctx.enter_context(tc.tile_pool(name="ppool", bufs=2))

        # ================= edge phase (per-layer schedule) ==================
        def edge_layer(layer, table, chunkx, pS, ROW, CH, NH, arl_t, out_cb,
                       groups, call_cols, colbase, SLOT_CAP, MSG_CAP):
            """layer: 1 or 2. CH: channels per head (32 / 64). NH: heads.
            arl_t: [128, T, NH] f32; pS: [128, T, NH] bf16 self-loop weights;
            chunkx: [CHUNK, ROW] own-rank projected rows (row 0 = sentinel).
            out_cb(t0, t1, unn, rec) per group.
            """
            HCL = CH * NH
            GMAX = max(t1 - t0 for (t0, t1, _, _) in groups)
            pair = table[:, :].rearrange("(n two) c -> n two c", two=2)
            lo_tab = pair[:, 0, :]      # even rows, stride 2*ROW
            hi_tab = pair[:, 1, :]      # odd rows, stride 2*ROW
            for gi, ((t0, t1, DL, DH), (lc0, lnc, hc0, hnc)) in enumerate(
                    zip(groups, call_cols)):
                G = t1 - t0
                SL, SH = G * DL, G * DH
                S = SL + SH
                g = gpool.tile([128, SLOT_CAP, ROW], BF16, tag="gbuf")
                nc.gpsimd.dma_gather(
                    g[:, 0:SL, :], lo_tab,
                    idx_s[:, colbase + lc0:colbase + lc0 + lnc],
                    num_idxs=SL * 128, num_idxs_reg=SL * 128,
                    elem_size=ROW, elem_step=2 * ROW, single_packet=False)
                nc.gpsimd.dma_gather(
                    g[:, SL:S, :], hi_tab,
                    idx_s[:, colbase + hc0:colbase + hc0 + hnc],
                    num_idxs=SH * 128, num_idxs_reg=SH * 128,
                    elem_size=ROW, elem_step=2 * ROW, single_packet=False,
                    queue_num=1)
                # own rows (self-loop h) for this group, plain dense DMA
                own = gpool.tile([128, GMAX, ROW], BF16, tag="own")
                nc.sync.dma_start(
                    own[:, 0:G, :],
                    chunkx[1 + t0 * 128:1 + t1 * 128, :].rearrange(
                        "(g p) c -> p g c", p=128))

                # pass A: logits + softmax numerators for both regions
                ps_r = []
                for ri, (off, D) in enumerate(((0, DL), (SL, DH))):
                    SD = G * D
                    gr = g[:, off:off + SD, :]
                    e = spool.tile([128, MSG_CAP, NH], F32, tag=f"e{ri}")
                    al = gr[:, :, HCL:HCL + 2 * NH].bitcast(F32)
                    e4 = e[:, 0:SD, :].rearrange("p (g d) h -> p g d h", g=G)
                    nc.vector.tensor_add(
                        e4, al.rearrange("p (g d) h -> p g d h", g=G),
                        _bc(arl_t[:, t0:t1, :].unsqueeze(2), (128, G, D, NH)))
                    nc.vector.scalar_tensor_tensor(
                        e[:, 0:SD, :], e[:, 0:SD, :], cfg.NEG, e[:, 0:SD, :],
                        op0=Alu.mult, op1=Alu.max)
                    p = spool.tile([128, MSG_CAP, NH], BF16, tag=f"p{ri}")
                    nc.scalar.activation(p[:, 0:SD, :], e[:, 0:SD, :], Act.Exp)
                    ps_r.append(p)

                # pass B: weight, aggregate
                parts, dens = [], []
                for ri, (off, D) in enumerate(((0, DL), (SL, DH))):
                    SD = G * D
                    gr = g[:, off:off + SD, :]
                    p = ps_r[ri]
                    den = spool.tile([128, GMAX, NH], F32, tag=f"den{ri}")
                    nc.vector.tensor_reduce(
                        den[:, 0:G, :],
                        p[:, 0:SD, :].rearrange("p (g d) h -> p g h d", g=G),
                        axis=mybir.AxisListType.X, op=Alu.add)
                    dens.append(den)
                    # msg <- p broadcast over c (doubling copies on the idle
                    # Scalar engine; last doubling absorbed into two
                    # half-multiplies on Vector), *= h
                    msg = ppool.tile([128, MSG_CAP, NH, CH], BF16, tag="msg")
                    nc.scalar.copy(msg[:, 0:SD, :, 0:1],
                                   p[:, 0:SD, :].unsqueeze(3))
                    half = CH // 2
                    k = 1
                    while k < half:
                        kk = min(k, half - k)
                        nc.scalar.copy(msg[:, 0:SD, :, k:k + kk],
                                       msg[:, 0:SD, :, 0:kk])
                        k += kk
                    gr4 = gr[:, :, 0:HCL].rearrange("p s (h c) -> p s h c",
                                                    h=NH)
                    msgh = msg[:, 0:SD, :, 0:half]
                    nc.vector.tensor_mul(msg[:, 0:SD, :, half:CH], msgh,
                                         gr4[:, :, :, half:CH])
                    nc.vector.tensor_mul(msgh, msgh, gr4[:, :, :, 0:half])
                    # tree-sum over slots within each tile -> [128, G, HCL]
                    msgt = msg[:, 0:SD, :, :].rearrange(
                        "p (g d) h c -> p g d (h c)", g=G)
                    part = apool.tile([128, GMAX, HCL], F32, tag=f"part{ri}")
                    cur = D
                    while cur > 2:
                        hh = cur // 2
                        nc.vector.tensor_add(
                            msgt[:, :, 0:hh, :], msgt[:, :, 0:hh, :],
                            msgt[:, :, cur - hh:cur, :])
                        cur -= hh
                    if cur == 2:
                        nc.vector.tensor_add(part[:, 0:G, :], msgt[:, :, 0, :],
                                             msgt[:, :, 1, :])
                    else:
                        nc.vector.tensor_copy(part[:, 0:G, :],
                                              msgt[:, :, 0, :])
                    parts.append(part)
                # self-loop contribution
                selfm = apool.tile([128, GMAX, NH, CH], F32, tag="selfm")
                nc.vector.tensor_mul(
                    selfm[:, 0:G, :, :],
                    own[:, 0:G, 0:HCL].rearrange("p g (h c) -> p g h c",
                                                 h=NH),
                    _bc(pS[:, t0:t1, :].unsqueeze(3), (128, G, NH, CH)))
                unn = apool.tile([128, GMAX, HCL], F32, tag="unn")
                nc.vector.tensor_add(unn[:, 0:G, :], parts[0][:, 0:G, :],
                                     parts[1][:, 0:G, :])
                nc.vector.tensor_add(
                    unn[:, 0:G, :], unn[:, 0:G, :],
                    selfm[:, 0:G, :, :].rearrange("p g h c -> p g (h c)"))
                den = spool.tile([128, GMAX, NH], F32, tag="dent")
                nc.vector.tensor_add(den[:, 0:G, :], dens[0][:, 0:G, :],
                                     dens[1][:, 0:G, :])
                nc.vector.tensor_add(den[:, 0:G, :], den[:, 0:G, :],
                                     pS[:, t0:t1, :])
                rec = spool.tile([128, GMAX, NH], F32, tag="rec")
                nc.vector.reciprocal(rec[:, 0:G, :], den[:, 0:G, :])
                out_cb(t0, t1, unn[:, 0:G, :], rec[:, 0:G, :])

        # ---- L1 epilogue: normalize, +b1, ELU, store h1 (per group);
        #      the L2 projection for the group's tiles runs inline so the
        #      first partial AG2 can fire while later L1 groups compute ----
        GMAX1 = max(t1 - t0 for (t0, t1, _, _) in sch.groups1)

        def l2_proj(t0, t1):
            for t in range(t0, t1):
                ps = psum.tile([128, OUT + 2], F32, tag="proj2")
                for k in range(KC):
                    nc.tensor.matmul(ps[:, :],
                                     h1T[:, k, t * 128:(t + 1) * 128],
                                     RHS2[:, k, :], start=(k == 0),
                                     stop=(k == KC - 1))
                nc.scalar.copy(tstage2[:, t, 0:OUT], ps[:, 0:OUT])
                al2_view = tstage2[:, t, OUT:OUT + 2].bitcast(F32)
                nc.vector.tensor_copy(al2_view[:, :], ps[:, OUT:OUT + 1])
                nc.vector.tensor_copy(ar2L[:, t, :], ps[:, OUT + 1:OUT + 2])
            nc.sync.dma_start(
                chunk2[1 + t0 * 128:1 + t1 * 128, :].rearrange(
                    "(t p) c -> p t c", p=128),
                tstage2[:, t0:t1, :])
            if sch.ta2 and t1 == sch.ta2:
                mA = 1 + sch.ta2 * 128
                nc.gpsimd.collective_compute(
                    "AllGather", Alu.bypass,
                    replica_groups=[list(range(cfg.R))],
                    ins=[chunk2[0:mA, :].opt()],
                    outs=[table2[0:cfg.R * mA, :].opt()])

        def l1_out(t0, t1, unn, rec):
            G = t1 - t0
            y = ypool.tile([128, GMAX1, H, HID], BF16, tag="y1")
            nc.vector.tensor_mul(
                y[:, 0:G, :, :],
                unn.rearrange("p g (h c) -> p g h c", h=H),
                _bc(rec.unsqueeze(3), (128, G, H, HID)))
            yf = y[:, 0:G, :, :].rearrange("p g h c -> p g (h c)")
            nc.vector.tensor_add(yf, yf, _bc(B1_s[:, :].unsqueeze(1),
                                             (128, G, HC1)))
            mn = ypool.tile([128, GMAX1, HC1], BF16, tag="mn1")
            nc.vector.tensor_scalar_min(mn[:, 0:G, :], yf, 0.0)
            nc.vector.tensor_scalar_max(yf, yf, 0.0)
            em = ypool.tile([128, GMAX1, HC1], BF16, tag="em1")
            nc.scalar.activation(em[:, 0:G, :], mn[:, 0:G, :], Act.Exp)
            h1t = ypool.tile([128, GMAX1, HC1], BF16, tag="h1t")
            nc.vector.scalar_tensor_tensor(h1t[:, 0:G, :], em[:, 0:G, :],
                                           -1.0, yf, op0=Alu.add, op1=Alu.add)
            nc.sync.dma_start(
                h1d[t0 * 128:t1 * 128, :].rearrange("(g p) c -> p g c", p=128),
                h1t[:, 0:G, :])
            for k in range(KC):
                nc.sync.dma_start_transpose(
                    h1T[:, k, t0 * 128:t1 * 128],
                    h1d[t0 * 128:t1 * 128, k * 128:(k + 1) * 128])
            l2_proj(t0, t1)

        h1T = epool.tile([128, KC, NT], BF16, tag="h1T")
        tstage2 = epool.tile([128, T, ROW2], BF16, tag="tstage2")
        nc.vector.memset(tstage2[:, :, :], 0.0)
        # sentinel row of chunk2 (covered by the first partial AG2)
        sent2e = epool.tile([1, ROW2], BF16, tag="sent2e")
        nc.vector.memset(sent2e[:, :], 0.0)
        nc.vector.memset(sent2e[:, OUT:OUT + 2].bitcast(F32), AL_SENT)
        nc.sync.dma_start(chunk2[0:1, :], sent2e[:, :])
        edge_layer(1, table1, chunk1, pS1, ROW1, HID, H, arL, l1_out,
                   sch.groups1, sch.call_cols1, 0, cfg.SLOT1, cfg.MSG1)

        eS2 = epool.tile([128, T, 1], F32, tag="eS2")
        al2L = tstage2[:, 0:T, OUT:OUT + 2].bitcast(F32)
        nc.vector.tensor_add(eS2[:, :, :], al2L, ar2L[:, :, :])
        nc.vector.scalar_tensor_tensor(
            eS2[:, :, :], eS2[:, :, :], cfg.NEG, eS2[:, :, :],
            op0=Alu.mult, op1=Alu.max)
        nc.scalar.activation(pS2[:, :, :], eS2[:, :, :], Act.Exp)
        dupb2 = epool.tile([128, NDUP // 128, ROW2], BF16, tag="dupb2")
        nc.gpsimd.dma_gather(
            dupb2[:, :, :], chunk2[0:1 + NT, :],
            dup_s[:, :],
            num_idxs=NDUP, num_idxs_reg=NDUP,
            elem_size=ROW2, single_packet=False, queue_num=1)
        nc.sync.dma_start(
            chunk2[1 + NT:1 + NT + NDUP, :].rearrange("(c p) w -> p c w", p=128),
            dupb2[:, :, :])
        # swap in the L2 gather-index section (overwrites L1's; the tile
        # framework orders this after the last L1 gather read)
        ncol2 = sch.offd - sch.off2
        nc.sync.dma_start(idx_s[:, 0:ncol2],
                          idxs_d[:, sch.off2:sch.off2 + ncol2])
        if sch.ta2:
            mA = 1 + sch.ta2 * 128
            mBp = CHUNK - mA + 1
            nc.gpsimd.collective_compute(
                "AllGather", Alu.bypass,
                replica_groups=[list(range(cfg.R))],
                ins=[chunk2[mA - 1:CHUNK, :].opt()],
                outs=[table2[cfg.R * mA:cfg.R * mA + cfg.R * mBp, :].opt()])
        else:
            nc.gpsimd.collective_compute(
                "AllGather", Alu.bypass,
                replica_groups=[list(range(cfg.R))],
                ins=[chunk2[0:CHUNK, :].opt()], outs=[table2[:, :].opt()])

        # ---- L2 epilogue: normalize, +b2, exp-sum; Ln deferred past loop ----
        ostage = epool.tile([128, T, OUT], F32, tag="ostage")
        ssumL = epool.tile([128, T, 1], F32, tag="ssumL")
        GMAX2 = max(t1 - t0 for (t0, t1, _, _) in sch.groups2)

        def l2_out(t0, t1, unn, rec):
            G = t1 - t0
            y = ypool.tile([128, GMAX2, OUT], F32, tag="y2")
            nc.vector.tensor_mul(y[:, 0:G, :], unn,
                                 _bc(rec, (128, G, OUT)))
            nc.vector.tensor_add(y[:, 0:G, :], y[:, 0:G, :],
                                 _bc(B2_s[:, :].unsqueeze(1), (128, G, OUT)))
            mx = spool.tile([128, GMAX2, 1], F32, tag="mx2")
            nc.vector.tensor_reduce(mx[:, 0:G, :], y[:, 0:G, :],
                                    axis=mybir.AxisListType.X, op=Alu.max)
            nc.vector.tensor_sub(ostage[:, t0:t1, :], y[:, 0:G, :],
                                 _bc(mx[:, 0:G, :], (128, G, OUT)))
            ex = spool.tile([128, GMAX2, OUT], F32, tag="ex2")
            nc.scalar.activation(ex[:, 0:G, :], ostage[:, t0:t1, :], Act.Exp)
            nc.vector.tensor_reduce(ssumL[:, t0:t1, :], ex[:, 0:G, :],
                                    axis=mybir.AxisListType.X, op=Alu.add)

        edge_layer(2, table2, chunk2, pS2, ROW2, OUT, 1, ar2L, l2_out,
                   sch.groups2, sch.call_cols2, 0, cfg.SLOT2, cfg.MSG2)
        lsL = epool.tile([128, T, 1], F32, tag="lsL")
        nc.scalar.activation(lsL[:, :, :], ssumL[:, :, :], Act.Ln)
        nc.vector.tensor_sub(ostage[:, :, :], ostage[:, :, :],
                             _bc(lsL[:, :, :], (128, T, OUT)))
        nc.sync.dma_start(out_d.ap().rearrange("(t p) c -> p t c", p=128),
                          ostage[:, :, :])

    nc.compile()
    return nc


def _host_inputs(cfg: Cfg, sch: Sched, inputs: dict):
    """Build per-rank in_maps from the full problem inputs."""
    x = np.asarray(inputs["x"], np.float32)
    W1 = np.asarray(inputs["W1"], np.float32)
    a1_src = np.asarray(inputs["a1_src"], np.float32)
    a1_dst = np.asarray(inputs["a1_dst"], np.float32)
    b1 = np.asarray(inputs["b1"], np.float32)
    W2 = np.asarray(inputs["W2"], np.float32)
    a2_src = np.asarray(inputs["a2_src"], np.float32)
    a2_dst = np.asarray(inputs["a2_dst"], np.float32)
    b2 = np.asarray(inputs["b2"], np.float32)
    H, HID, HC1, OUT = cfg.HEADS, cfg.HID, cfg.HC1, cfg.OUT

    # block-diagonal per-head attention matrices: al = h @ A1s
    A1s = np.zeros((HC1, H), np.float32)
    A1d = np.zeros((HC1, H), np.float32)
    for h in range(H):
        A1s[h * HID:(h + 1) * HID, h] = a1_src[h]
        A1d[h * HID:(h + 1) * HID, h] = a1_dst[h]

    common = {
        "W1": np.ascontiguousarray(W1),
        "W1T": np.ascontiguousarray(W1.T),
        "A1s": A1s, "A1d": A1d,
        "B1rep": np.tile(b1[None, :], (128, 1)).astype(np.float32),
        "W2": np.ascontiguousarray(W2),
        "W2T": np.ascontiguousarray(W2.T),
        "a2s": np.ascontiguousarray(a2_src.reshape(OUT, 1)),
        "a2d": np.ascontiguousarray(a2_dst.reshape(OUT, 1)),
        "B2rep": np.tile(b2[None, :], (128, 1)).astype(np.float32),
    }
    in_maps = []
    for r in range(cfg.R):
        m = dict(common)
        xp = np.zeros((cfg.NT, x.shape[1]), np.float32)
        xp[:cfg.NPR] = x[sch.perm[r]]
        m["xT"] = np.ascontiguousarray(xp.T)
        m["idxs"] = np.ascontiguousarray(sch.idx16[r])
        in_maps.append(m)
    return in_maps


def run(cfg: Cfg, inputs: dict, trace: bool = False, tmpdir: str | None = None):
    edge_index = np.asarray(inputs["edge_index"])
    # self-loops are handled densely in-kernel; only real edges are gathered
    src = edge_index[0].astype(np.int64)
    dst = edge_index[1].astype(np.int64)

    sch = build_schedule(cfg, src, dst)
    nc = build_program(cfg, sch)
    in_maps = _host_inputs(cfg, sch, inputs)
    res = bass_utils.run_bass_kernel_spmd(
        nc, in_maps, core_ids=list(range(cfg.R)), trace=trace, tmpdir=tmpdir)
    out = np.empty((cfg.N, cfg.OUT), np.float32)
    for r in range(cfg.R):
        o = res.results[r]["out"]
        out[sch.perm[r]] = o[:cfg.NPR]
    return out, res


def kernel(**inputs) -> np.ndarray:
    cfg = Cfg()
    out, _ = run(cfg, inputs)
    return out


if __name__ == "__main__":
    import reference
    inputs = {k: np.asarray(v) for k, v in reference.setup_inputs().items()}
    out = kernel(**inputs)
    exp = np.asarray(reference.reference(**reference.setup_inputs()))
    err = np.abs(out - exp).max() / (np.abs(exp).max() + 1e-12)
    print("rel err:", err)


# revision 27
# speedup vs baseline: 1.0348x; 1.0339x over previous
"""2-layer GAT (gnn_message_passing) on 8 TRN2 NeuronCores.

Strategy (graph/data parallel, per sharding hint):
  - Nodes are partitioned across 8 ranks (6250 dst nodes each). Each rank owns
    the segment-softmax + aggregation for its destination nodes.
  - Per layer, every rank computes the projected features (h = x @ W,
    attention source/dest logits al/ar fused into the same matmul via an
    augmented RHS) for ITS OWN nodes, writes them as rows of a gather table
    (768B rows for layer 1: 256 bf16 h + 8 f32 al; 256B rows for layer 2),
    then an AllGather replicates the full table to every rank.
  - Edge stage: destinations are degree-sorted and packed into tiles of 128
    (dst on partitions); consecutive tiles are grouped with a UNIFORM padded
    slot count per group so that the whole group is one rectangular grid
    [128, G, D, ...] and every vector/scalar op covers the full group in a
    single instruction. Source rows are fetched with dma_gather (SWDGE
    indexed gather). Since gather indices are int16 (max 32767) and the
    table has ~60k rows, rows are addressed through an even/odd pair view
    (idx = row//2): each group issues one even-window and one odd-window
    gather, so every edge needs a parity side.
  - Slot-padding control: per-tile slot counts are max(c_even), max(c_odd)
    over the (degree-sorted) tile's dsts. A host-side balancer chooses each
    source node's row parity to balance every dst's in-edge parity split;
    residual "blocker" sources are DUPLICATED (their row is copied once more
    at the opposite parity at the end of the owner rank's chunk), making all
    their edges per-edge flexible. This brings padded slots within ~2% of
    the true floor (max in-degree per tile). L1 and L2 use separate group
    schedules: L2 rows are 4x smaller so its groups merge ~3x more tiles.
  - Segment softmax is all free-dim math: e = leakyrelu(al_src + ar_dst) on
    the slot grid, p = exp(e) (no max-subtract needed at these magnitudes;
    mathematically identical), denom = free-dim reduce, normalization applied
    AFTER aggregation (divide the aggregated sums by denom).
  - Aggregation: msg = p (broadcast over channels by doubling copies on the
    otherwise-idle Scalar engine) * h_src, then a pairwise tree of wide
    tensor adds along the slot dim.
  - Padding slots read a sentinel table row (h = 0, al = -1e30 -> p = 0).

The full output is assembled on the host from the 8 per-rank outputs
(undoing the degree-sort permutation).
"""

import sys
from contextlib import ExitStack
from dataclasses import dataclass

import numpy as np

for _p in ("/opt/trn_rl_repo",):
    if _p not in sys.path:
        sys.path.insert(0, _p)

import concourse.bass as bass
import concourse.bacc as bacc
import concourse.mybir as mybir
import concourse.tile as tile
from concourse import bass_utils


F32 = mybir.dt.float32
BF16 = mybir.dt.bfloat16
I16 = mybir.dt.int16
AL_SENT = -1.0e30
Alu = mybir.AluOpType
Act = mybir.ActivationFunctionType


@dataclass
class Cfg:
    N: int = 50000
    E: int = 500000          # edges before self-loops
    F_IN: int = 128
    HID: int = 32
    HEADS: int = 8
    OUT: int = 64
    NEG: float = 0.2
    R: int = 8
    SLOT1: int = 36          # L1 max uniform slots per gather group
    MSG1: int = 28           # L1 max slots per region
    SLOT2: int = 104         # L2 caps (rows 4x smaller; pools shared with L1)
    MSG2: int = 96
    NDUP: int = 1664         # duplicate rows per rank (multiple of 128, even)
    TA1: int = 0             # AG1 split tile boundary (0 = no split)
    TA2: int = 37            # AG2 split tile boundary (0 = no split)

    @property
    def HC1(self):
        return self.HEADS * self.HID     # 256

    @property
    def NPR(self):
        return self.N // self.R

    @property
    def T(self):
        return (self.NPR + 127) // 128   # dst tiles per rank

    @property
    def NT(self):
        return self.T * 128

    @property
    def CHUNK(self):
        # sentinel + staged rows (NT >= NPR) + duplicate rows; must be odd
        return 1 + self.NT + self.NDUP

    @property
    def TROWS(self):
        return self.R * self.CHUNK

    @property
    def ROW1(self):
        return 384                       # bf16 elems: 256 h + 16 (8xf32 al) + pad

    @property
    def ROW2(self):
        return 128                       # bf16 elems: 64 h2 + 2 (1xf32 al2) + pad


@dataclass
class Sched:
    perm: np.ndarray          # [R, NPR] perm[r][pos] = global node id
    groups1: list             # L1 groups: (t0, t1, DL, DH)
    call_cols1: list          # per group, section-relative (lo0, lnc, hi0, hnc)
    groups2: list             # L2 groups
    call_cols2: list
    idx16: np.ndarray         # [R, 128, TOTCOL] int16 (L1 | L2 | dup)
    off2: int                 # column offset of the L2 section
    offd: int                 # column offset of the dup section
    ta2: int = 0              # AG2 split tile boundary (0 = no split)
    sent_hi2: int = 0         # odd-pad sentinel idx for the L2 table
    kdup: int = 0             # dup slots [0,kdup) sourced from tiles < TA2


def _pack_idx(vals: np.ndarray) -> np.ndarray:
    """int32 row-idx values -> the [128, n/16] int16 SWDGE index layout."""
    assert vals.min() >= 0 and vals.max() < 32768, (vals.min(), vals.max())
    return np.tile(vals.astype(np.int16).reshape(-1, 16).T, (8, 1))


def build_schedule(cfg: Cfg, src: np.ndarray, dst: np.ndarray) -> Sched:
    N, R, NPR, T = cfg.N, cfg.R, cfg.NPR, cfg.T
    CHUNK, NT = cfg.CHUNK, cfg.NT
    assert CHUNK % 2 == 1 and 4 * CHUNK < 32768
    deg = np.bincount(dst, minlength=N).astype(np.int64)

    # ---- global degree-sorted tiles (1024 nodes per global tile) ----
    gorder = np.argsort(-deg, kind="stable")
    gtile = np.empty(N, np.int64)
    for t in range(T):
        gtile[gorder[t * 1024:(t + 1) * 1024]] = t
    tile_of_dst = gtile[dst]
    maxdeg_t = np.array([max(1, deg[gorder[t * 1024:(t + 1) * 1024]].max())
                         for t in range(T)])

    eorder = np.argsort(src, kind="stable")
    s_sorted = src[eorder]
    d_sorted = dst[eorder]
    starts = np.searchsorted(s_sorted, np.arange(N + 1))

    # ---- parity balancing: conflict-free vectorized greedy ----
    rng = np.random.default_rng(12345)
    parity = np.zeros(N, np.int8)
    tile_nodes = []
    for t in range(T):
        nodes = gorder[t * 1024:(t + 1) * 1024]
        tile_nodes.append(nodes)
        p = np.zeros(len(nodes), np.int8)
        p[:len(nodes) // 2] = 1
        rng.shuffle(p)
        parity[nodes] = p

    c_e = np.zeros(N, np.int32)
    c_o = np.zeros(N, np.int32)
    pe = parity[src]
    np.add.at(c_e, dst[pe == 0], 1)
    np.add.at(c_o, dst[pe == 1], 1)

    # alternate the ceil side per tile so parity peaks (and hence dup-copy
    # parity demand) split ~evenly between the even and odd windows
    ceil_half = np.ceil((maxdeg_t + 1) / 2).astype(np.int64)
    Te = np.where(np.arange(T) % 2 == 0, ceil_half, (maxdeg_t + 1) - ceil_half)
    To = (maxdeg_t + 1) - Te
    TeD = Te[tile_of_dst]
    ToD = To[tile_of_dst]
    imb = np.zeros(T, np.int64)
    CAP, W = 12, 8.0

    def pen(c, Tt):
        return np.where(c > Tt, W ** np.minimum(c - Tt, 6), 0.0)

    for rnd in range(120):
        ceD = c_e[dst]
        coD = c_o[dst]
        d_eo = (pen(ceD - 1, TeD) - pen(ceD, TeD)) + (pen(coD + 1, ToD) - pen(coD, ToD))
        d_oe = (pen(coD - 1, ToD) - pen(coD, ToD)) + (pen(ceD + 1, TeD) - pen(ceD, TeD))
        cum_eo = np.concatenate([[0.], np.cumsum(d_eo[eorder])])
        cum_oe = np.concatenate([[0.], np.cumsum(d_oe[eorder])])
        g_eo = -(cum_eo[starts[1:]] - cum_eo[starts[:-1]])
        g_oe = -(cum_oe[starts[1:]] - cum_oe[starts[:-1]])
        gain = np.where(parity == 0, g_eo, g_oe)
        cand = np.where(gain > 1e-9)[0]
        if len(cand) == 0:
            break
        cand = cand[np.argsort(-gain[cand])]
        dirty = np.zeros(N, bool)
        napp = 0
        for u in cand:
            ds = d_sorted[starts[u]:starts[u + 1]]
            if dirty[ds].any():
                continue
            t = gtile[u]
            delta = 1 if parity[u] == 0 else -1
            if abs(imb[t] + delta) > CAP:
                continue
            dirty[ds] = True
            imb[t] += delta
            napp += 1
            if parity[u] == 0:
                c_e[ds] -= 1
                c_o[ds] += 1
                parity[u] = 1
            else:
                c_o[ds] -= 1
                c_e[ds] += 1
                parity[u] = 0
        if napp == 0:
            break
    # repair per-tile parity balance to exact 50/50
    for t in range(T):
        while imb[t] != 0:
            nodes = tile_nodes[t]
            want = 1 if imb[t] > 0 else 0
            pool = nodes[parity[nodes] == want]
            ceD = c_e[dst]
            coD = c_o[dst]
            if want == 1:
                dpe = (pen(coD - 1, ToD) - pen(coD, ToD)) + (pen(ceD + 1, TeD) - pen(ceD, TeD))
            else:
                dpe = (pen(ceD - 1, TeD) - pen(ceD, TeD)) + (pen(coD + 1, ToD) - pen(coD, ToD))
            cum = np.concatenate([[0.], np.cumsum(dpe[eorder])])
            gg = -(cum[starts[1:]] - cum[starts[:-1]])
            bu = pool[np.argmax(gg[pool])]
            ds = d_sorted[starts[bu]:starts[bu + 1]]
            if want == 1:
                c_o[ds] -= 1
                c_e[ds] += 1
                parity[bu] = 0
                imb[t] -= 1
            else:
                c_e[ds] -= 1
                c_o[ds] += 1
                parity[bu] = 1
                imb[t] += 1

    # ---- duplicate "blocker" sources until forced maxima reach the floor ----
    dup = np.zeros(N, bool)
    max_dups = (cfg.NDUP // 2 - 32) * 2 * R  # conservative global budget

    def forced_stats():
        f_e = np.zeros(N, np.int32)
        f_o = np.zeros(N, np.int32)
        m = ~dup[src]
        pp = parity[src]
        np.add.at(f_e, dst[m & (pp == 0)], 1)
        np.add.at(f_o, dst[m & (pp == 1)], 1)
        FE = np.zeros(T, np.int64)
        FO = np.zeros(T, np.int64)
        np.maximum.at(FE, tile_of_dst, f_e[dst])
        np.maximum.at(FO, tile_of_dst, f_o[dst])
        return f_e, f_o, np.maximum(FE, 1), np.maximum(FO, 1)

    for it in range(200):
        f_e, f_o, FE, FO = forced_stats()
        bind = (FE + FO) > maxdeg_t
        if not bind.any() or dup.sum() >= max_dups:
            break
        peak_e = (bind[tile_of_dst] & (f_e[dst] == FE[tile_of_dst])
                  & (parity[src] == 0) & ~dup[src])
        peak_o = (bind[tile_of_dst] & (f_o[dst] == FO[tile_of_dst])
                  & (parity[src] == 1) & ~dup[src])
        sc = np.zeros(N, np.int64)
        np.add.at(sc, src[peak_e | peak_o], 1)
        order = np.argsort(-sc)
        take = order[sc[order] > 0][:200]
        if len(take) == 0:
            break
        dup[take] = True
    f_e, f_o, FE, FO = forced_stats()

    # ---- per-tile slot budgets + flexible (dup-sourced) edge assignment ----
    B_t = np.maximum(FE + FO, maxdeg_t)
    mid = np.where(np.arange(T) % 2 == 0, np.ceil(B_t / 2),
                   np.floor(B_t / 2)).astype(np.int64)
    Te_t = np.clip(mid, FE, B_t - FO)
    flex_cnt = np.zeros(N, np.int32)
    np.add.at(flex_cnt, dst[dup[src]], 1)
    TeN = Te_t[gtile]                        # per-dst lo budget
    lo_cnt = f_e + np.minimum(flex_cnt, np.maximum(0, TeN - f_e)).astype(np.int32)
    assert (lo_cnt <= TeN).all()
    assert ((deg - lo_cnt) <= (B_t - Te_t)[gtile]).all()

    # ---- placement: assign nodes to (rank, position) honoring parity ----
    perm = np.empty((R, NPR), np.int64)
    rank_of = np.empty(N, np.int64)
    sortpos = np.empty(N, np.int64)
    for t in range(T):
        nodes = tile_nodes[t]
        k = len(nodes) // R
        # dup'd nodes first within each parity class, then stride-8 interleave
        # across ranks so each rank gets an equal share of dup copies
        ev_nodes = nodes[parity[nodes] == 0]
        od_nodes = nodes[parity[nodes] == 1]
        ev_nodes = ev_nodes[np.argsort(~dup[ev_nodes], kind="stable")]
        od_nodes = od_nodes[np.argsort(~dup[od_nodes], kind="stable")]
        for r in range(R):
            pos = t * 128 + np.arange(k)
            pp = (r + 1 + pos) % 2            # row parity of each position
            ev_pos = pos[pp == 0]
            od_pos = pos[pp == 1]
            a = ev_nodes[r::R]
            b = od_nodes[r::R]
            assert len(a) == len(ev_pos) and len(b) == len(od_pos), (t, r)
            perm[r, ev_pos] = a
            perm[r, od_pos] = b
            rank_of[a] = r
            rank_of[b] = r
            sortpos[a] = ev_pos
            sortpos[b] = od_pos

    row_of = rank_of * CHUNK + 1 + sortpos
    assert ((row_of % 2) == parity).all()

    # ---- dup row placement: copy at opposite parity in owner rank chunk ----
    dup_row = np.full(N, -1, np.int64)
    dup_vals = np.zeros((R, cfg.NDUP), np.int64)
    kdup = cfg.NDUP
    for r in range(R):
        dn = np.where(dup & (rank_of == r))[0]
        need_even = dn[parity[dn] == 1]
        need_odd = dn[parity[dn] == 0]
        # dups sourced from tiles < TA2 first: their chunk rows are ready
        # early, so their copy can be prefetched before the last L1 group
        isB_e = (sortpos[need_even] >= cfg.TA2 * 128).astype(np.int8)
        isB_o = (sortpos[need_odd] >= cfg.TA2 * 128).astype(np.int8)
        need_even = need_even[np.argsort(isB_e, kind="stable")]
        need_odd = need_odd[np.argsort(isB_o, kind="stable")]
        nA = 2 * min(int((isB_e == 0).sum()), int((isB_o == 0).sum()))
        kdup = min(kdup, nA)
        slots = np.arange(cfg.NDUP)
        spar = (r + 1 + NT + slots) % 2
        ev_slots = slots[spar == 0]
        od_slots = slots[spar == 1]
        assert len(need_even) <= len(ev_slots) and len(need_odd) <= len(od_slots), (
            r, len(need_even), len(need_odd))
        for nn, ss in ((need_even, ev_slots), (need_odd, od_slots)):
            rows = r * CHUNK + 1 + NT + ss[:len(nn)]
            dup_row[nn] = rows
            dup_vals[r, ss[:len(nn)]] = 1 + sortpos[nn]
    row_even = np.where(parity == 0, row_of, dup_row)
    row_odd = np.where(parity == 1, row_of, dup_row)

    # ---- per-edge side + slot assignment ----
    dkey = rank_of[dst] * NPR + sortpos[dst]
    side = np.where(dup[src], -1, parity[src]).astype(np.int64)  # -1 = flex
    order0 = np.lexsort((side, dkey))
    s_side = side[order0]
    s_dkey = dkey[order0]
    cnt = np.bincount(dkey, minlength=R * NPR)
    start = np.concatenate([[0], np.cumsum(cnt)])[:-1]
    pos_in = np.arange(len(order0)) - start[s_dkey]
    nflo = np.bincount(dkey[side == 0], minlength=R * NPR)
    lo_cnt_d = lo_cnt[dst[order0]]
    flex_lo_quota = lo_cnt_d - nflo[s_dkey]
    is_flex = s_side == -1
    new_side = np.where(is_flex, np.where(pos_in < flex_lo_quota, 0, 1), s_side)

    order1 = np.lexsort((new_side, s_dkey))
    f_side = new_side[order1]
    f_dkey = s_dkey[order1]
    f_edge = order0[order1]
    pos_f = np.arange(len(order1)) - start[f_dkey]
    nlo_f = np.bincount(f_dkey[f_side == 0], minlength=R * NPR)
    slot = np.where(f_side == 0, pos_f, pos_f - nlo_f[f_dkey])
    f_src = src[f_edge]
    f_row = np.where(f_side == 0, row_even[f_src], row_odd[f_src])
    assert (f_row >= 0).all()
    assert ((f_row % 2) == f_side).all()

    # per-tile slot maxima (shared by all ranks)
    D_lo = np.zeros(T, np.int64)
    D_hi = np.zeros(T, np.int64)
    nlo_g = nlo_f.reshape(R, NPR)
    nhi_g = (cnt - nlo_f).reshape(R, NPR)
    for t in range(T):
        sl = slice(t * 128, min((t + 1) * 128, NPR))
        D_lo[t] = max(1, nlo_g[:, sl].max())
        D_hi[t] = max(1, nhi_g[:, sl].max())

    def make_groups(SLOT_CAP, MSG_CAP):
        groups = []
        t0 = 0
        while t0 < T:
            t1, DL, DH = t0, 0, 0
            while t1 < T:
                nDL = max(DL, int(D_lo[t1]))
                nDH = max(DH, int(D_hi[t1]))
                G1 = t1 - t0 + 1
                if (G1 * (nDL + nDH) > SLOT_CAP
                        or G1 * max(nDL, nDH) > MSG_CAP):
                    break
                DL, DH, t1 = nDL, nDH, t1 + 1
            assert t1 > t0, (t0, D_lo[t0], D_hi[t0])
            groups.append((t0, t1, DL, DH))
            t0 = t1
        return groups

    def layout(groups):
        """Section-relative column layout. Returns (call_cols, pos_base, ncols)."""
        call_cols = []
        pos_base = np.zeros((T, 2), np.int64)
        col = 0
        for (t0, t1, DL, DH) in groups:
            G = t1 - t0
            lo0 = col
            for g, t in enumerate(range(t0, t1)):
                pos_base[t, 0] = col * 16 + g * DL * 128
            col += G * DL * 8
            hi0 = col
            for g, t in enumerate(range(t0, t1)):
                pos_base[t, 1] = col * 16 + g * DH * 128
            col += G * DH * 8
            call_cols.append((lo0, G * DL * 8, hi0, G * DH * 8))
        return call_cols, pos_base, col

    groups1 = make_groups(cfg.SLOT1, cfg.MSG1)
    groups2 = make_groups(cfg.SLOT2, cfg.MSG2)

    # ---- L2 table row mapping (two-piece layout for the split AllGather) ----
    # piece A = all ranks' chunk rows [0, mA); piece B = chunk rows
    # [mA-1, CHUNK) per rank (leading duplicate of row mA-1 keeps mB' odd so
    # that row parity matches the chunk parity used for side assignment).
    if cfg.TA2:
        ta2 = 0
        for (g_t0, g_t1, _dl, _dh) in groups1:
            if g_t1 >= cfg.TA2:
                ta2 = g_t1
                break
        mA = 1 + ta2 * 128
        mBp = CHUNK - mA + 1
        assert mA % 2 == 1 and mBp % 2 == 1

        def row2_of(c, r):
            return np.where(c < mA, r * mA + c,
                            R * mA + r * mBp + c - mA + 1)
        sent_hi2 = mA // 2  # rank-1 sentinel: row2 = mA (odd)
    else:
        ta2 = 0

        def row2_of(c, r):
            return r * CHUNK + c
        sent_hi2 = (CHUNK - 1) // 2
    chunk_row = row_of - rank_of * CHUNK
    row2_node = row2_of(chunk_row, rank_of)
    dup_chunk_row = np.where(dup_row >= 0, dup_row % CHUNK, 0)
    dup_row2 = np.where(dup_row >= 0,
                        row2_of(dup_chunk_row, rank_of), -1)
    assert ((row2_node % 2) == parity).all()
    assert ((dup_row2[dup] % 2) == (1 - parity[dup])).all()
    row_even2 = np.where(parity == 0, row2_node, dup_row2)
    row_odd2 = np.where(parity == 1, row2_node, dup_row2)
    f_row2 = np.where(f_side == 0, row_even2[f_src], row_odd2[f_src])
    assert (f_row2 >= 0).all() and ((f_row2 % 2) == f_side).all()
    call_cols1, pos_base1, ncol1 = layout(groups1)
    call_cols2, pos_base2, ncol2 = layout(groups2)
    ndup_cols = cfg.NDUP // 16
    TOTCOL = ncol1 + ncol2 + ndup_cols

    SENT_LO = 0                 # even pad: rank-0 sentinel row 0
    SENT_HI = (CHUNK - 1) // 2  # odd pad: rank-1 sentinel row CHUNK (odd)

    e_rank = f_dkey // NPR
    e_pos = f_dkey % NPR
    e_tile = e_pos // 128
    e_part = e_pos % 128
    idx16 = np.empty((R, 128, TOTCOL), np.int16)
    for r in range(R):
        m = (e_rank == r)
        mlo = m & (f_side == 0)
        mhi = m & (f_side == 1)
        sections = []
        for (groups, call_cols, pos_base, ncol, rows, s_hi) in (
                (groups1, call_cols1, pos_base1, ncol1, f_row, SENT_HI),
                (groups2, call_cols2, pos_base2, ncol2, f_row2, sent_hi2)):
            vals = np.empty(ncol * 16, np.int32)
            for (lc0, lnc, hc0, hnc) in call_cols:
                vals[lc0 * 16:(lc0 + lnc) * 16] = SENT_LO
                vals[hc0 * 16:(hc0 + hnc) * 16] = s_hi
            p_lo = pos_base[e_tile[mlo], 0] + slot[mlo] * 128 + e_part[mlo]
            vals[p_lo] = rows[mlo] // 2
            p_hi = pos_base[e_tile[mhi], 1] + slot[mhi] * 128 + e_part[mhi]
            vals[p_hi] = (rows[mhi] - 1) // 2
            sections.append(_pack_idx(vals))
        sections.append(_pack_idx(dup_vals[r]))
        idx16[r] = np.concatenate(sections, axis=1)

    kdup = (kdup // 128) * 128
    return Sched(perm=perm, groups1=groups1, call_cols1=call_cols1,
                 groups2=groups2, call_cols2=call_cols2, idx16=idx16,
                 off2=ncol1, offd=ncol1 + ncol2, ta2=ta2, sent_hi2=sent_hi2,
                 kdup=kdup)


def _bc(ap, shape):
    """broadcast an AP to shape (step-0 dims)"""
    return ap.broadcast_to(list(shape))


def build_program(cfg: Cfg, sch: Sched):
    """Build the single SPMD Bass program. Returns nc."""
    nc = bacc.Bacc("TRN2", target_bir_lowering=False, debug=False,
                   num_devices=cfg.R, num_swdge_queues=2)
    T, NPR, CHUNK, TROWS, NT = cfg.T, cfg.NPR, cfg.CHUNK, cfg.TROWS, cfg.NT
    HC1, H, HID, OUT = cfg.HC1, cfg.HEADS, cfg.HID, cfg.OUT
    ROW1, ROW2, NDUP = cfg.ROW1, cfg.ROW2, cfg.NDUP
    TOTCOL = sch.idx16.shape[2]

    # ---- I/O ----
    xT = nc.dram_tensor("xT", [cfg.F_IN, NT], F32, kind="ExternalInput")
    idxs_d = nc.dram_tensor("idxs", [128, TOTCOL], I16, kind="ExternalInput")
    W1_d = nc.dram_tensor("W1", [cfg.F_IN, HC1], F32, kind="ExternalInput")
    W1T_d = nc.dram_tensor("W1T", [HC1, cfg.F_IN], F32, kind="ExternalInput")
    A1s_d = nc.dram_tensor("A1s", [HC1, H], F32, kind="ExternalInput")
    A1d_d = nc.dram_tensor("A1d", [HC1, H], F32, kind="ExternalInput")
    B1_d = nc.dram_tensor("B1rep", [128, HC1], F32, kind="ExternalInput")
    W2_d = nc.dram_tensor("W2", [HC1, OUT], F32, kind="ExternalInput")
    W2T_d = nc.dram_tensor("W2T", [OUT, HC1], F32, kind="ExternalInput")
    a2s_d = nc.dram_tensor("a2s", [OUT, 1], F32, kind="ExternalInput")
    a2d_d = nc.dram_tensor("a2d", [OUT, 1], F32, kind="ExternalInput")
    B2_d = nc.dram_tensor("B2rep", [128, OUT], F32, kind="ExternalInput")
    out_d = nc.dram_tensor("out", [NT, OUT], F32, kind="ExternalOutput")

    KC = HC1 // 128   # contraction chunks over HC1 (2)

    with tile.TileContext(nc) as tc, ExitStack() as ctx:
        dram = ctx.enter_context(tc.tile_pool(name="dram", bufs=1, space="DRAM"))
        const = ctx.enter_context(tc.tile_pool(name="const", bufs=1))
        psum = ctx.enter_context(tc.tile_pool(name="psum", bufs=2, space="PSUM"))

        # DRAM scratch
        chunk1 = dram.tile([CHUNK, ROW1], BF16)
        table1 = dram.tile([TROWS, ROW1], BF16, addr_space="Shared")
        chunk2 = dram.tile([CHUNK, ROW2], BF16)
        # table2 is NOT in the Shared address space: Shared DRAM allows a
        # single writer instruction, and the AG2 overlap needs two partial
        # AllGathers writing disjoint (contiguous) row ranges of the
        # two-piece layout (one extra row per rank in piece B)
        TROWS2 = cfg.R * (CHUNK + 1) if sch.ta2 else TROWS
        table2 = dram.tile([TROWS2, ROW2], BF16,
                           addr_space="Shared" if not sch.ta2 else "Local")
        h1d = dram.tile([NT, HC1], BF16)

        # ---- persistent constants ----
        # idx SBUF is swapped per layer: [active-layer cols | dup cols]
        MAXCOL = max(sch.off2, sch.offd - sch.off2)
        idx_s = const.tile([128, MAXCOL], I16, tag="idx")
        nc.sync.dma_start(idx_s[:, 0:sch.off2], idxs_d[:, 0:sch.off2])
        dup_s = const.tile([128, NDUP // 16], I16, tag="idxdup")
        nc.sync.dma_start(dup_s[:, :], idxs_d[:, sch.offd:sch.offd + NDUP // 16])
        RHS1 = const.tile([128, HC1 + 2 * H], F32, tag="rhs1")
        nc.sync.dma_start(RHS1[:, 0:HC1], W1_d[:, :])
        B1_s = const.tile([128, HC1], BF16, tag="b1")
        B1f_s = const.tile([128, HC1], F32, tag="b1f")
        nc.sync.dma_start(B1f_s[:, :], B1_d[:, :])
        nc.vector.tensor_copy(B1_s[:, :], B1f_s[:, :])
        B2_s = const.tile([128, OUT], F32, tag="b2")
        nc.sync.dma_start(B2_s[:, :], B2_d[:, :])
        arL = const.tile([128, T, H], F32, tag="arL")
        nc.vector.memset(arL[:, :, :], 0.0)
        ar2L = const.tile([128, T, 1], F32, tag="ar2L")
        nc.vector.memset(ar2L[:, :, :], 0.0)
        RHS2 = const.tile([128, KC, OUT + 2], BF16, tag="rhs2")
        nc.gpsimd.dma_start(RHS2[:, :, 0:OUT],
                            W2_d.ap().rearrange("(k p) c -> p k c", p=128))
        pS1 = const.tile([128, T, H], BF16, tag="pS1")
        pS2 = const.tile([128, T, 1], BF16, tag="pS2")

        # ================= phase 1: projection + table 1 ====================
        with tc.tile_pool(name="ph1", bufs=1) as ph1:
            xT_s = ph1.tile([128, NT], F32, tag="xT")
            nc.sync.dma_start(xT_s[:, :], xT[:, :])
            W1T_s = ph1.tile([128, KC, 128], F32, tag="w1t")
            nc.sync.dma_start(W1T_s[:, :, :],
                              W1T_d.ap().rearrange("(k p) f -> p k f", p=128))
            A1s_s = ph1.tile([128, KC, H], F32, tag="a1s")
            nc.sync.dma_start(A1s_s[:, :, :],
                              A1s_d.ap().rearrange("(k p) h -> p k h", p=128))
            A1d_s = ph1.tile([128, KC, H], F32, tag="a1d")
            nc.sync.dma_start(A1d_s[:, :, :],
                              A1d_d.ap().rearrange("(k p) h -> p k h", p=128))
            W2T_s = ph1.tile([OUT, HC1], F32, tag="w2t")
            nc.sync.dma_start(W2T_s[:, :], W2T_d[:, :])
            a2s_s = ph1.tile([OUT, 1], F32, tag="a2s")
            nc.sync.dma_start(a2s_s[:, :], a2s_d[:, :])
            a2d_s = ph1.tile([OUT, 1], F32, tag="a2d")
            nc.sync.dma_start(a2d_s[:, :], a2d_d[:, :])

            # fold attention vectors into projection RHS
            for (dst_off, A_s) in ((HC1, A1s_s), (HC1 + H, A1d_s)):
                ps = psum.tile([128, H], F32, tag="wprep")
                for k in range(KC):
                    nc.tensor.matmul(ps[:, :], W1T_s[:, k, :], A_s[:, k, :],
                                     start=(k == 0), stop=(k == KC - 1))
                nc.vector.tensor_copy(RHS1[:, dst_off:dst_off + H], ps[:, :])
            for (dst_off, a_s) in ((OUT, a2s_s), (OUT + 1, a2d_s)):
                for k in range(KC):
                    ps = psum.tile([128, 1], F32, tag="wprep2")
                    nc.tensor.matmul(ps[:, :], W2T_s[:, k * 128:(k + 1) * 128],
                                     a_s[:, :], start=True, stop=True)
                    nc.vector.tensor_copy(RHS2[:, k, dst_off:dst_off + 1],
                                          ps[:, :])

            # sentinel row -> chunk row 0 (h = 0, al = -1e30); written first
            # so the first partial AllGather can cover it
            sent1 = ph1.tile([1, ROW1], BF16, tag="sent1")
            nc.vector.memset(sent1[:, :], 0.0)
            nc.vector.memset(sent1[:, HC1:HC1 + 2 * H].bitcast(F32), AL_SENT)
            nc.sync.dma_start(chunk1[0:1, :], sent1[:, :])
            tstage = ph1.tile([128, T, ROW1], BF16, tag="tstage1")
            nc.vector.memset(tstage[:, :, :], 0.0)
            TA1 = cfg.TA1
            for t in range(T):
                ps = psum.tile([128, HC1 + 2 * H], F32, tag="proj1")
                nc.tensor.matmul(ps[:, :], xT_s[:, t * 128:(t + 1) * 128],
                                 RHS1[:, :], start=True, stop=True)
                nc.scalar.copy(tstage[:, t, 0:HC1], ps[:, 0:HC1])
                al_view = tstage[:, t, HC1:HC1 + 2 * H].bitcast(F32)
                nc.vector.tensor_copy(al_view[:, :], ps[:, HC1:HC1 + H])
                nc.vector.tensor_copy(arL[:, t, :],
                                      ps[:, HC1 + H:HC1 + 2 * H])
                if TA1 and t == TA1 - 1:
                    nc.sync.dma_start(
                        chunk1[1:1 + TA1 * 128, :].rearrange(
                            "(t p) c -> p t c", p=128),
                        tstage[:, 0:TA1, :])
                    nc.gpsimd.collective_compute(
                        "AllGather", Alu.bypass,
                        replica_groups=[list(range(cfg.R))],
                        ins=[chunk1[0:1 + TA1 * 128, :].opt()],
                        outs=[table1[:, :].rearrange(
                            "(r c) w -> r c w", r=cfg.R)[
                            :, 0:1 + TA1 * 128, :].opt()])
            # dense self-loop weights: pS1 = exp(leakyrelu(al + ar))
            eS = ph1.tile([128, T, H], F32, tag="eS")
            alL = tstage[:, 0:T, HC1:HC1 + 2 * H].bitcast(F32)
            nc.vector.tensor_add(eS[:, :, :], alL, arL[:, :, :])

            nc.vector.scalar_tensor_tensor(
                eS[:, :, :], eS[:, :, :], cfg.NEG, eS[:, :, :],
                op0=Alu.mult, op1=Alu.max)
            nc.scalar.activation(pS1[:, :, :], eS[:, :, :], Act.Exp)
            TA1 = cfg.TA1
            nc.sync.dma_start(
                chunk1[1 + TA1 * 128:1 + NT, :].rearrange(
                    "(t p) c -> p t c", p=128),
                tstage[:, TA1:T, :])
            # duplicate rows: indexed re-fetch of own chunk rows, append
            dupb = ph1.tile([128, NDUP // 128, ROW1], BF16, tag="dupb")
            nc.gpsimd.dma_gather(
                dupb[:, :, :], chunk1[0:1 + NT, :],
                dup_s[:, :],
                num_idxs=NDUP, num_idxs_reg=NDUP,
                elem_size=ROW1, single_packet=False, queue_num=1)
            nc.sync.dma_start(
                chunk1[1 + NT:1 + NT + NDUP, :].rearrange(
                    "(c p) w -> p c w", p=128),
                dupb[:, :, :])
        row1b = 1 + cfg.TA1 * 128 if cfg.TA1 else 0
        nc.gpsimd.collective_compute(
            "AllGather", Alu.bypass,
            replica_groups=[list(range(cfg.R))],
            ins=[chunk1[row1b:CHUNK, :].opt()],
            outs=[table1[:, :].rearrange("(r c) w -> r c w", r=cfg.R)[
                :, row1b:CHUNK, :].opt()])

        epool = ctx.enter_context(tc.tile_pool(name="edge", bufs=1))
        gpool = ctx.enter_context(tc.tile_pool(name="gpool", bufs=2))
        spool = ctx.enter_context(tc.tile_pool(name="spool", bufs=2))
        apool = ctx.enter_context(tc.tile_pool(name="apool", bufs=1))
        ypool = ctx.enter_context(tc.tile_pool(name="ypool", bufs=2))
        ppool = ctx.enter_context(tc.tile_pool(name="ppool", bufs=2))

        # ================= edge phase (per-layer schedule) ==================
        def edge_layer(layer, table, chunkx, pS, ROW, CH, NH, arl_t, out_cb,
                       groups, call_cols, colbase, SLOT_CAP, MSG_CAP):
            """layer: 1 or 2. CH: channels per head (32 / 64). NH: heads.
            arl_t: [128, T, NH] f32; pS: [128, T, NH] bf16 self-loop weights;
            chunkx: [CHUNK, ROW] own-rank projected rows (row 0 = sentinel).
            out_cb(t0, t1, unn, rec) per group.
            """
            HCL = CH * NH
            GMAX = max(t1 - t0 for (t0, t1, _, _) in groups)
            pair = table[:, :].rearrange("(n two) c -> n two c", two=2)
            lo_tab = pair[:, 0, :]      # even rows, stride 2*ROW
            hi_tab = pair[:, 1, :]      # odd rows, stride 2*ROW
            for gi, ((t0, t1, DL, DH), (lc0, lnc, hc0, hnc)) in enumerate(
                    zip(groups, call_cols)):
                G = t1 - t0
                SL, SH = G * DL, G * DH
                S = SL + SH
                g = gpool.tile([128, SLOT_CAP, ROW], BF16, tag="gbuf")
                nc.gpsimd.dma_gather(
                    g[:, 0:SL, :], lo_tab,
                    idx_s[:, colbase + lc0:colbase + lc0 + lnc],
                    num_idxs=SL * 128, num_idxs_reg=SL * 128,
                    elem_size=ROW, elem_step=2 * ROW, single_packet=False)
                nc.gpsimd.dma_gather(
                    g[:, SL:S, :], hi_tab,
                    idx_s[:, colbase + hc0:colbase + hc0 + hnc],
                    num_idxs=SH * 128, num_idxs_reg=SH * 128,
                    elem_size=ROW, elem_step=2 * ROW, single_packet=False,
                    queue_num=1)
                # own rows (self-loop h) for this group, plain dense DMA
                own = gpool.tile([128, GMAX, ROW], BF16, tag="own")
                nc.sync.dma_start(
                    own[:, 0:G, :],
                    chunkx[1 + t0 * 128:1 + t1 * 128, :].rearrange(
                        "(g p) c -> p g c", p=128))

                # pass A: logits + softmax numerators for both regions
                ps_r = []
                for ri, (off, D) in enumerate(((0, DL), (SL, DH))):
                    SD = G * D
                    gr = g[:, off:off + SD, :]
                    e = spool.tile([128, MSG_CAP, NH], F32, tag=f"e{ri}")
                    al = gr[:, :, HCL:HCL + 2 * NH].bitcast(F32)
                    e4 = e[:, 0:SD, :].rearrange("p (g d) h -> p g d h", g=G)
                    nc.vector.tensor_add(
                        e4, al.rearrange("p (g d) h -> p g d h", g=G),
                        _bc(arl_t[:, t0:t1, :].unsqueeze(2), (128, G, D, NH)))
                    nc.vector.scalar_tensor_tensor(
                        e[:, 0:SD, :], e[:, 0:SD, :], cfg.NEG, e[:, 0:SD, :],
                        op0=Alu.mult, op1=Alu.max)
                    p = spool.tile([128, MSG_CAP, NH], BF16, tag=f"p{ri}")
                    nc.scalar.activation(p[:, 0:SD, :], e[:, 0:SD, :], Act.Exp)
                    ps_r.append(p)

                # pass B: weight, aggregate
                parts, dens = [], []
                for ri, (off, D) in enumerate(((0, DL), (SL, DH))):
                    SD = G * D
                    gr = g[:, off:off + SD, :]
                    p = ps_r[ri]
                    den = spool.tile([128, GMAX, NH], F32, tag=f"den{ri}")
                    nc.vector.tensor_reduce(
                        den[:, 0:G, :],
                        p[:, 0:SD, :].rearrange("p (g d) h -> p g h d", g=G),
                        axis=mybir.AxisListType.X, op=Alu.add)
                    dens.append(den)
                    # msg <- p broadcast over c (doubling copies on the idle
                    # Scalar engine; last doubling absorbed into two
                    # half-multiplies on Vector), *= h
                    msg = ppool.tile([128, MSG_CAP, NH, CH], BF16, tag="msg")
                    nc.scalar.copy(msg[:, 0:SD, :, 0:1],
                                   p[:, 0:SD, :].unsqueeze(3))
                    half = CH // 2
                    k = 1
                    while k < half:
                        kk = min(k, half - k)
                        nc.scalar.copy(msg[:, 0:SD, :, k:k + kk],
                                       msg[:, 0:SD, :, 0:kk])
                        k += kk
                    gr4 = gr[:, :, 0:HCL].rearrange("p s (h c) -> p s h c",
                                                    h=NH)
                    msgh = msg[:, 0:SD, :, 0:half]
                    nc.vector.tensor_mul(msg[:, 0:SD, :, half:CH], msgh,
                                         gr4[:, :, :, half:CH])
                    nc.vector.tensor_mul(msgh, msgh, gr4[:, :, :, 0:half])
                    # tree-sum over slots within each tile -> [128, G, HCL]
                    msgt = msg[:, 0:SD, :, :].rearrange(
                        "p (g d) h c -> p g d (h c)", g=G)
                    part = apool.tile([128, GMAX, HCL], F32, tag=f"part{ri}")
                    cur = D
                    while cur > 2:
                        hh = cur // 2
                        nc.vector.tensor_add(
                            msgt[:, :, 0:hh, :], msgt[:, :, 0:hh, :],
                            msgt[:, :, cur - hh:cur, :])
                        cur -= hh
                    if cur == 2:
                        nc.vector.tensor_add(part[:, 0:G, :], msgt[:, :, 0, :],
                                             msgt[:, :, 1, :])
                    else:
                        nc.vector.tensor_copy(part[:, 0:G, :],
                                              msgt[:, :, 0, :])
                    parts.append(part)
                # self-loop contribution
                selfm = apool.tile([128, GMAX, NH, CH], F32, tag="selfm")
                nc.vector.tensor_mul(
                    selfm[:, 0:G, :, :],
                    own[:, 0:G, 0:HCL].rearrange("p g (h c) -> p g h c",
                                                 h=NH),
                    _bc(pS[:, t0:t1, :].unsqueeze(3), (128, G, NH, CH)))
                unn = apool.tile([128, GMAX, HCL], F32, tag="unn")
                nc.vector.tensor_add(unn[:, 0:G, :], parts[0][:, 0:G, :],
                                     parts[1][:, 0:G, :])
                nc.vector.tensor_add(
                    unn[:, 0:G, :], unn[:, 0:G, :],
                    selfm[:, 0:G, :, :].rearrange("p g h c -> p g (h c)"))
                den = spool.tile([128, GMAX, NH], F32, tag="dent")
                nc.vector.tensor_add(den[:, 0:G, :], dens[0][:, 0:G, :],
                                     dens[1][:, 0:G, :])
                nc.vector.tensor_add(den[:, 0:G, :], den[:, 0:G, :],
                                     pS[:, t0:t1, :])
                rec = spool.tile([128, GMAX, NH], F32, tag="rec")
                nc.vector.reciprocal(rec[:, 0:G, :], den[:, 0:G, :])
                out_cb(t0, t1, unn[:, 0:G, :], rec[:, 0:G, :])

        # ---- L1 epilogue: normalize, +b1, ELU, store h1 (per group);
        #      the L2 projection for the group's tiles runs inline so the
        #      first partial AG2 can fire while later L1 groups compute ----
        GMAX1 = max(t1 - t0 for (t0, t1, _, _) in sch.groups1)

        def l2_proj(t0, t1):
            for t in range(t0, t1):
                ps = psum.tile([128, OUT + 2], F32, tag="proj2")
                for k in range(KC):
                    nc.tensor.matmul(ps[:, :],
                                     h1T[:, k, t * 128:(t + 1) * 128],
                                     RHS2[:, k, :], start=(k == 0),
                                     stop=(k == KC - 1))
                nc.scalar.copy(tstage2[:, t, 0:OUT], ps[:, 0:OUT])
                al2_view = tstage2[:, t, OUT:OUT + 2].bitcast(F32)
                nc.vector.tensor_copy(al2_view[:, :], ps[:, OUT:OUT + 1])
                nc.vector.tensor_copy(ar2L[:, t, :], ps[:, OUT + 1:OUT + 2])
            nc.sync.dma_start(
                chunk2[1 + t0 * 128:1 + t1 * 128, :].rearrange(
                    "(t p) c -> p t c", p=128),
                tstage2[:, t0:t1, :])
            if sch.ta2 and t1 == sch.ta2:
                mA = 1 + sch.ta2 * 128
                nc.gpsimd.collective_compute(
                    "AllGather", Alu.bypass,
                    replica_groups=[list(range(cfg.R))],
                    ins=[chunk2[0:mA, :].opt()],
                    outs=[table2[0:cfg.R * mA, :].opt()])
                if KD:
                    nc.gpsimd.dma_gather(
                        dupb2[:, 0:KD // 128, :],
                        chunk2[0:1 + cfg.TA2 * 128, :],
                        dup_s[:, 0:KD // 16],
                        num_idxs=KD, num_idxs_reg=KD,
                        elem_size=ROW2, single_packet=False, queue_num=1)
                    nc.sync.dma_start(
                        chunk2[1 + NT:1 + NT + KD, :].rearrange(
                            "(c p) w -> p c w", p=128),
                        dupb2[:, 0:KD // 128, :])

        def l1_out(t0, t1, unn, rec):
            G = t1 - t0
            y = ypool.tile([128, GMAX1, H, HID], BF16, tag="y1")
            nc.vector.tensor_mul(
                y[:, 0:G, :, :],
                unn.rearrange("p g (h c) -> p g h c", h=H),
                _bc(rec.unsqueeze(3), (128, G, H, HID)))
            yf = y[:, 0:G, :, :].rearrange("p g h c -> p g (h c)")
            nc.vector.tensor_add(yf, yf, _bc(B1_s[:, :].unsqueeze(1),
                                             (128, G, HC1)))
            mn = ypool.tile([128, GMAX1, HC1], BF16, tag="mn1")
            nc.vector.tensor_scalar_min(mn[:, 0:G, :], yf, 0.0)
            nc.vector.tensor_scalar_max(yf, yf, 0.0)
            em = ypool.tile([128, GMAX1, HC1], BF16, tag="em1")
            nc.scalar.activation(em[:, 0:G, :], mn[:, 0:G, :], Act.Exp)
            h1t = ypool.tile([128, GMAX1, HC1], BF16, tag="h1t")
            nc.vector.scalar_tensor_tensor(h1t[:, 0:G, :], em[:, 0:G, :],
                                           -1.0, yf, op0=Alu.add, op1=Alu.add)
            nc.sync.dma_start(
                h1d[t0 * 128:t1 * 128, :].rearrange("(g p) c -> p g c", p=128),
                h1t[:, 0:G, :])
            for k in range(KC):
                nc.sync.dma_start_transpose(
                    h1T[:, k, t0 * 128:t1 * 128],
                    h1d[t0 * 128:t1 * 128, k * 128:(k + 1) * 128])
            l2_proj(t0, t1)

        h1T = epool.tile([128, KC, NT], BF16, tag="h1T")
        tstage2 = epool.tile([128, T, ROW2], BF16, tag="tstage2")
        nc.vector.memset(tstage2[:, :, :], 0.0)
        KD = sch.kdup
        dupb2 = epool.tile([128, NDUP // 128, ROW2], BF16, tag="dupb2")
        # sentinel row of chunk2 (covered by the first partial AG2)
        sent2e = epool.tile([1, ROW2], BF16, tag="sent2e")
        nc.vector.memset(sent2e[:, :], 0.0)
        nc.vector.memset(sent2e[:, OUT:OUT + 2].bitcast(F32), AL_SENT)
        nc.sync.dma_start(chunk2[0:1, :], sent2e[:, :])
        edge_layer(1, table1, chunk1, pS1, ROW1, HID, H, arL, l1_out,
                   sch.groups1, sch.call_cols1, 0, cfg.SLOT1, cfg.MSG1)

        eS2 = epool.tile([128, T, 1], F32, tag="eS2")
        al2L = tstage2[:, 0:T, OUT:OUT + 2].bitcast(F32)
        nc.vector.tensor_add(eS2[:, :, :], al2L, ar2L[:, :, :])
        nc.vector.scalar_tensor_tensor(
            eS2[:, :, :], eS2[:, :, :], cfg.NEG, eS2[:, :, :],
            op0=Alu.mult, op1=Alu.max)
        nc.scalar.activation(pS2[:, :, :], eS2[:, :, :], Act.Exp)
        KR = NDUP - KD
        nc.gpsimd.dma_gather(
            dupb2[:, KD // 128:NDUP // 128, :], chunk2[0:1 + NT, :],
            dup_s[:, KD // 16:NDUP // 16],
            num_idxs=KR, num_idxs_reg=KR,
            elem_size=ROW2, single_packet=False, queue_num=1)
        nc.sync.dma_start(
            chunk2[1 + NT + KD:1 + NT + NDUP, :].rearrange(
                "(c p) w -> p c w", p=128),
            dupb2[:, KD // 128:NDUP // 128, :])
        # swap in the L2 gather-index section (overwrites L1's; the tile
        # framework orders this after the last L1 gather read)
        ncol2 = sch.offd - sch.off2
        nc.sync.dma_start(idx_s[:, 0:ncol2],
                          idxs_d[:, sch.off2:sch.off2 + ncol2])
        if sch.ta2:
            mA = 1 + sch.ta2 * 128
            mBp = CHUNK - mA + 1
            nc.gpsimd.collective_compute(
                "AllGather", Alu.bypass,
                replica_groups=[list(range(cfg.R))],
                ins=[chunk2[mA - 1:CHUNK, :].opt()],
                outs=[table2[cfg.R * mA:cfg.R * mA + cfg.R * mBp, :].opt()])
        else:
            nc.gpsimd.collective_compute(
                "AllGather", Alu.bypass,
                replica_groups=[list(range(cfg.R))],
                ins=[chunk2[0:CHUNK, :].opt()], outs=[table2[:, :].opt()])

        # ---- L2 epilogue: normalize, +b2, exp-sum; Ln deferred past loop ----
        ostage = epool.tile([128, T, OUT], F32, tag="ostage")
        ssumL = epool.tile([128, T, 1], F32, tag="ssumL")
        GMAX2 = max(t1 - t0 for (t0, t1, _, _) in sch.groups2)

        def l2_out(t0, t1, unn, rec):
            G = t1 - t0
            y = ypool.tile([128, GMAX2, OUT], F32, tag="y2")
            nc.vector.tensor_mul(y[:, 0:G, :], unn,
                                 _bc(rec, (128, G, OUT)))
            nc.vector.tensor_add(y[:, 0:G, :], y[:, 0:G, :],
                                 _bc(B2_s[:, :].unsqueeze(1), (128, G, OUT)))
            mx = spool.tile([128, GMAX2, 1], F32, tag="mx2")
            nc.vector.tensor_reduce(mx[:, 0:G, :], y[:, 0:G, :],
                                    axis=mybir.AxisListType.X, op=Alu.max)
            nc.vector.tensor_sub(ostage[:, t0:t1, :], y[:, 0:G, :],
                                 _bc(mx[:, 0:G, :], (128, G, OUT)))
            ex = spool.tile([128, GMAX2, OUT], F32, tag="ex2")
            nc.scalar.activation(ex[:, 0:G, :], ostage[:, t0:t1, :], Act.Exp)
            nc.vector.tensor_reduce(ssumL[:, t0:t1, :], ex[:, 0:G, :],
                                    axis=mybir.AxisListType.X, op=Alu.add)

        edge_layer(2, table2, chunk2, pS2, ROW2, OUT, 1, ar2L, l2_out,
                   sch.groups2, sch.call_cols2, 0, cfg.SLOT2, cfg.MSG2)
        lsL = epool.tile([128, T, 1], F32, tag="lsL")
        nc.scalar.activation(lsL[:, :, :], ssumL[:, :, :], Act.Ln)
        nc.vector.tensor_sub(ostage[:, :, :], ostage[:, :, :],
                             _bc(lsL[:, :, :], (128, T, OUT)))
        nc.sync.dma_start(out_d.ap().rearrange("(t p) c -> p t c", p=128),
                          ostage[:, :, :])

    nc.compile()
    return nc


def _host_inputs(cfg: Cfg, sch: Sched, inputs: dict):
    """Build per-rank in_maps from the full problem inputs."""
    x = np.asarray(inputs["x"], np.float32)
    W1 = np.asarray(inputs["W1"], np.float32)
    a1_src = np.asarray(inputs["a1_src"], np.float32)
    a1_dst = np.asarray(inputs["a1_dst"], np.float32)
    b1 = np.asarray(inputs["b1"], np.float32)
    W2 = np.asarray(inputs["W2"], np.float32)
    a2_src = np.asarray(inputs["a2_src"], np.float32)
    a2_dst = np.asarray(inputs["a2_dst"], np.float32)
    b2 = np.asarray(inputs["b2"], np.float32)
    H, HID, HC1, OUT = cfg.HEADS, cfg.HID, cfg.HC1, cfg.OUT

    # block-diagonal per-head attention matrices: al = h @ A1s
    A1s = np.zeros((HC1, H), np.float32)
    A1d = np.zeros((HC1, H), np.float32)
    for h in range(H):
        A1s[h * HID:(h + 1) * HID, h] = a1_src[h]
        A1d[h * HID:(h + 1) * HID, h] = a1_dst[h]

    common = {
        "W1": np.ascontiguousarray(W1),
        "W1T": np.ascontiguousarray(W1.T),
        "A1s": A1s, "A1d": A1d,
        "B1rep": np.tile(b1[None, :], (128, 1)).astype(np.float32),
        "W2": np.ascontiguousarray(W2),
        "W2T": np.ascontiguousarray(W2.T),
        "a2s": np.ascontiguousarray(a2_src.reshape(OUT, 1)),
        "a2d": np.ascontiguousarray(a2_dst.reshape(OUT, 1)),
        "B2rep": np.tile(b2[None, :], (128, 1)).astype(np.float32),
    }
    in_maps = []
    for r in range(cfg.R):
        m = dict(common)
        xp = np.zeros((cfg.NT, x.shape[1]), np.float32)
        xp[:cfg.NPR] = x[sch.perm[r]]
        m["xT"] = np.ascontiguousarray(xp.T)
        m["idxs"] = np.ascontiguousarray(sch.idx16[r])
        in_maps.append(m)
    return in_maps


def run(cfg: Cfg, inputs: dict, trace: bool = False, tmpdir: str | None = None):
    edge_index = np.asarray(inputs["edge_index"])
    # self-loops are handled densely in-kernel; only real edges are gathered
    src = edge_index[0].astype(np.int64)
    dst = edge_index[1].astype(np.int64)

    sch = build_schedule(cfg, src, dst)
    nc = build_program(cfg, sch)
    in_maps = _host_inputs(cfg, sch, inputs)
    res = bass_utils.run_bass_kernel_spmd(
        nc, in_maps, core_ids=list(range(cfg.R)), trace=trace, tmpdir=tmpdir)
    out = np.empty((cfg.N, cfg.OUT), np.float32)
    for r in range(cfg.R):
        o = res.results[r]["out"]
        out[sch.perm[r]] = o[:cfg.NPR]
    return out, res


def kernel(**inputs) -> np.ndarray:
    cfg = Cfg()
    out, _ = run(cfg, inputs)
    return out


if __name__ == "__main__":
    import reference
    inputs = {k: np.asarray(v) for k, v in reference.setup_inputs().items()}
    out = kernel(**inputs)
    exp = np.asarray(reference.reference(**reference.setup_inputs()))
    err = np.abs(out - exp).max() / (np.abs(exp).max() + 1e-12)
    print("rel err:", err)
